# revision 65
# baseline (speedup 1.0000x reference)
"""Trainium2 Bass kernel for a pre-norm transformer block (MHSA + FFN).

Sharding: 8 cores, data parallel over (batch, seq-half). Core c handles
batch c//2, sequence half c%2. Inputs are permuted so each core's own
1024 tokens come first; attention K/V run over all 2048 tokens of the
batch (softmax is permutation invariant).

Numerics: ALL five projections (Q/K/V/Wo/W1/W2) run as 3-term
compensated fp8 DoubleRow (hi=e4m3, lo=e5m2, x@W ~= xh@Wh + xh@Wl +
xl@Wh, 4x bf16 matmul throughput per term), with weights pre-scaled x32
and split host-side; the normalized activations are split on-chip after
the TensorE transpose. Scores stay f32r (softmax logits are ~N(0,26^2)
— direct fp8 there flips argmaxes); softmax probs in bf16 with a
constant exp shift; PV runs probs-stationary so only the 65-wide
(dh+denominator) V operand streams. LayerNorm gains/biases are folded
into the downstream weights/biases host-side, so on-chip LN is pure
z-normalization.

Scheduling: the softmax exp (the largest non-matmul cost, ~218us if
ACT-only) is split ~60/40 between ACT's native Exp and a Schraudolph
exp2 (DVE int mul-add + Pool max/clamp; GPSIMD cannot read PSUM, so
the PSUM-side op must be the DVE one). Stage A (LN1+transpose+split)
streams pairs 0-3's Q/K/V matmuls in as each transposed 512-token
block completes, so the PE is busy during LN; DMA issue order is
arranged so the first x tiles are not stuck behind weight/bias loads
on the serial DMA lane.

Wo and W2 both run token-major (lhsT = the already-feature-major oT /
h1), so the attention and FFN outputs land directly in token order:
no aoT/ffnT intermediates, no transpose-back matmuls, and the
residual + bias ride the single DVE evacuation of each PSUM group
(bo is pre-added to the x residual host-side, b2 comes in replicated
across partitions). x2 is kept in bf16 to fit the full-width W2 tiles
in SBUF. PV safety: a chain-completion tracker drains each QKV
generator fully before the first PV that contracts over its V tiles.
"""
import contextlib

import numpy as np
import ml_dtypes

import concourse.bass as bass
import concourse.tile as tile
import concourse.mybir as mybir
from concourse.bass_utils import run_bass_kernel_spmd
from concourse.masks import make_identity

B, T, C = 4, 2048, 1024
H, DH = 16, 64
DFF = 4 * C
N_CORES = 8
TQ = T // 2          # tokens owned per core
TS = T               # key/value tokens per core
NKO = C // 128       # 8 contraction tiles for C
F32R = mybir.dt.float32r
F32 = mybir.dt.float32
BF16 = mybir.dt.bfloat16
E4 = mybir.dt.float8e4
E5 = mybir.dt.float8e5
EXP_BIAS = -128.0
EPS = 1e-5
WSCALE = 32.0
DR = mybir.MatmulPerfMode.DoubleRow

# ---------------------------------------------------------------------------
# Compat: this walrus build accepts at most 1 sem-wait per regular
# instruction (2 per InstEventSemaphore). bacc misses some tile-generated
# instructions, so split waits ourselves after finalize.
_ev_counter = [0]


def _legalize_sem_waits(nc):
    for func in nc.m.functions:
        for bb in func.blocks:
            new = []
            changed = False
            for inst in bb.instructions:
                si = inst.sync_info
                cap = 2 if isinstance(inst, mybir.InstEventSemaphore) else 1
                if si is not None and len(si.on_wait) > cap:
                    waits = list(si.on_wait)
                    for i in range(cap, len(waits), 2):
                        _ev_counter[0] += 1
                        e = mybir.InstEventSemaphore(
                            name=f"EVSPLIT-{_ev_counter[0]}", ins=[], outs=[])
                        e.engine = inst.engine
                        e.sync_info = mybir.SyncInfo(
                            on_wait=waits[i:i + 2], on_update=[])
                        new.append(e)
                    inst.sync_info = mybir.SyncInfo(
                        on_wait=waits[:cap], on_update=list(si.on_update))
                    changed = True
                new.append(inst)
            if changed:
                bb.instructions = new


# ---------------------------------------------------------------------------

def _ln_stats_a(nc, stats, x_ap, eps_t):
    """bn stats + sqrt(var+eps) for x_ap [128, C]; returns (mv, rstd)."""
    st = stats.tile([128, 2, 6], F32, tag="bnstats")
    mv = stats.tile([128, 2], F32, tag="bnaggr")
    xg = x_ap.rearrange("p (s d) -> p s d", s=2)
    for s in range(2):
        nc.vector.bn_stats(out=st[:, s, :], in_=xg[:, s, :])
    nc.vector.bn_aggr(out=mv[:], in_=st[:])
    rstd = stats.tile([128, 1], F32, tag="rstd")
    nc.scalar.activation(out=rstd[:], in_=mv[:, 1:2],
                         func=mybir.ActivationFunctionType.Sqrt,
                         bias=eps_t[:], scale=1.0)
    return mv, rstd


def _ln_stats_b(nc, mv, rstd, x_ap, out_ap, pool=False):
    """finish z-normalize: recip + (x - mu) * rstd."""
    nc.vector.reciprocal(out=rstd[:], in_=rstd[:])
    eng = nc.gpsimd if pool else nc.vector
    eng.tensor_scalar(out=out_ap, in0=x_ap,
                      scalar1=mv[:, 0:1], scalar2=rstd[:],
                      op0=mybir.AluOpType.subtract,
                      op1=mybir.AluOpType.mult)


def _bcast0(ap, free):
    """Broadcast a [128, n] AP along a new stride-0 free dim of size `free`."""
    return bass.AP(tensor=ap.tensor, offset=ap.offset,
                   ap=[list(d) for d in ap.ap] + [[0, free]])


def _build_nc():
    nc = bass.Bass()

    # ---- I/O ----
    x_d = nc.dram_tensor("x", [T, C], F32, kind="ExternalInput")
    xpb_d = nc.dram_tensor("xpb", [TQ, C], F32, kind="ExternalInput")
    wqh_d = nc.dram_tensor("wqh", [C, C], E4, kind="ExternalInput")
    wql_d = nc.dram_tensor("wql", [C, C], E5, kind="ExternalInput")
    wkh_d = nc.dram_tensor("wkh", [C, C], E4, kind="ExternalInput")
    wkl_d = nc.dram_tensor("wkl", [C, C], E5, kind="ExternalInput")
    wvh_d = nc.dram_tensor("wvh", [C, C], E4, kind="ExternalInput")
    wvl_d = nc.dram_tensor("wvl", [C, C], E5, kind="ExternalInput")
    woh_d = nc.dram_tensor("woh", [C, C], E4, kind="ExternalInput")
    wol_d = nc.dram_tensor("wol", [C, C], E5, kind="ExternalInput")
    w1h_d = nc.dram_tensor("w1h", [C, DFF], E4, kind="ExternalInput")
    w1l_d = nc.dram_tensor("w1l", [C, DFF], E5, kind="ExternalInput")
    w2h_d = nc.dram_tensor("w2h", [DFF, C], E4, kind="ExternalInput")
    w2l_d = nc.dram_tensor("w2l", [DFF, C], E5, kind="ExternalInput")
    bq_d = nc.dram_tensor("bq", [C], F32, kind="ExternalInput")
    bk_d = nc.dram_tensor("bk", [C], F32, kind="ExternalInput")
    bv_d = nc.dram_tensor("bv", [128, C], BF16, kind="ExternalInput")
    bo_d = nc.dram_tensor("bo", [C], F32, kind="ExternalInput")
    b1_d = nc.dram_tensor("b1", [DFF], F32, kind="ExternalInput")
    b2_d = nc.dram_tensor("b2", [C], F32, kind="ExternalInput")
    b2r_d = nc.dram_tensor("b2r", [128, C], F32, kind="ExternalInput")
    out_d = nc.dram_tensor("out", [TQ, C], F32, kind="ExternalOutput")

    wqh_r = wqh_d.rearrange("(o p) f -> p o f", p=128)
    wql_r = wql_d.rearrange("(o p) f -> p o f", p=128)
    wkh_r = wkh_d.rearrange("(o p) f -> p o f", p=128)
    wkl_r = wkl_d.rearrange("(o p) f -> p o f", p=128)
    wvh_r = wvh_d.rearrange("(o p) f -> p o f", p=128)
    wvl_r = wvl_d.rearrange("(o p) f -> p o f", p=128)

    with tile.TileContext(nc) as tc:
        with contextlib.ExitStack() as top:
            consts = top.enter_context(tc.tile_pool(name="consts", bufs=1))
            ps = top.enter_context(tc.tile_pool(name="ps", bufs=2, space="PSUM"))
            stats = top.enter_context(tc.tile_pool(name="stats", bufs=8))

            ident_b = consts.tile([128, 128], BF16, tag="identb")
            make_identity(nc, ident_b)
            ident_r = consts.tile([128, 128], F32R, tag="identr")
            nc.vector.tensor_copy(out=ident_r[:], in_=ident_b[:])
            ebias = consts.tile([128, 1], F32, tag="ebias")
            nc.vector.memset(ebias[:], EXP_BIAS)
            eps_t = consts.tile([128, 1], F32, tag="eps")
            nc.vector.memset(eps_t[:], EPS)
            bq_s = consts.tile([128, NKO], F32, tag="bq")
            bk_s = consts.tile([128, NKO], F32, tag="bk")
            bo_s = consts.tile([128, NKO], F32, tag="bo")
            b2_s = consts.tile([128, NKO], F32, tag="b2")
            b1_s = consts.tile([128, DFF // 128], F32, tag="b1")
            bv_r = consts.tile([128, C], BF16, tag="bvr")

            def load_biases():
                for dst, src_ in ((bq_s, bq_d), (bk_s, bk_d), (bo_s, bo_d),
                                  (b2_s, b2_d), (b1_s, b1_d)):
                    nc.sync.dma_start(out=dst[:],
                                      in_=src_.rearrange("(o p) -> p o", p=128))
                nc.sync.dma_start(out=bv_r[:], in_=bv_d[:, :])

            # ============ Stages A-C: LN1, QKV, attention ============
            with contextlib.ExitStack() as abc:
                xnp = abc.enter_context(tc.tile_pool(name="xnp", bufs=8))
                xnT_hi = [xnp.tile([128, NKO, 512], E4, tag="xnThi",
                                   name=f"xnThi{i}") for i in range(4)]
                xnT_lo = [xnp.tile([128, NKO, 512], E5, tag="xnTlo",
                                   name=f"xnTlo{i}") for i in range(4)]
                wgp = abc.enter_context(tc.tile_pool(name="wgp", bufs=2))
                qkp = abc.enter_context(tc.tile_pool(name="qkp", bufs=2))
                vgp = abc.enter_context(tc.tile_pool(name="vgp", bufs=2))

                qk_tiles = {}
                vg_tiles = {}
                wq_tiles = {}
                wv_tiles = {}

                def load_qk_w(g):
                    wqt_h = wgp.tile([128, NKO, 256], E4, tag="wqth")
                    wqt_l = wgp.tile([128, NKO, 256], E5, tag="wqtl")
                    wkt_h = wgp.tile([128, NKO, 256], E4, tag="wkth")
                    wkt_l = wgp.tile([128, NKO, 256], E5, tag="wktl")
                    fsl_w = slice(g * 256, (g + 1) * 256)
                    nc.sync.dma_start(out=wqt_h[:], in_=wqh_r[:, :, fsl_w])
                    nc.sync.dma_start(out=wqt_l[:], in_=wql_r[:, :, fsl_w])
                    nc.sync.dma_start(out=wkt_h[:], in_=wkh_r[:, :, fsl_w])
                    nc.sync.dma_start(out=wkt_l[:], in_=wkl_r[:, :, fsl_w])
                    wq_tiles[g] = (wqt_h, wqt_l, wkt_h, wkt_l)

                def load_v_w(g):
                    wvt_h = wgp.tile([128, NKO, 256], E4, tag="wvth")
                    wvt_l = wgp.tile([128, NKO, 256], E5, tag="wvtl")
                    nc.sync.dma_start(out=wvt_h[:], in_=wvh_r[:, :, g * 256:(g + 1) * 256])
                    nc.sync.dma_start(out=wvt_l[:], in_=wvl_r[:, :, g * 256:(g + 1) * 256])
                    wv_tiles[g] = (wvt_h, wvt_l)
                    vg = vgp.tile([128, TS // 128, 4, 65], BF16, tag="vg")
                    vg_tiles[g] = vg
                    nc.vector.memset(vg[:, :, :, DH:DH + 1], 1.0)

                def alloc_qk(pair):
                    i = pair % 2
                    qps = [qkp.tile([128, 512], F32R, tag=f"qp{i}c{ch}",
                                    name=f"qp{pair}c{ch}")
                           for ch in range(TQ // 512)]
                    kps = [qkp.tile([128, 512], F32R, tag=f"kp{i}c{ch}",
                                    name=f"kp{pair}c{ch}")
                           for ch in range(TS // 512)]
                    qk_tiles[pair] = (qps, kps)

                def qk_group(pair, kind, ch, on_act=False):
                    """One Q or K psum group (3-term fp8 DR) + evacuation."""
                    g, i = pair // 2, pair % 2
                    wqt_h, wqt_l, wkt_h, wkt_l = wq_tiles[g]
                    wh, wl = (wqt_h, wqt_l) if kind == 'q' else (wkt_h, wkt_l)
                    dst = qk_tiles[pair][0 if kind == 'q' else 1][ch]
                    bias = bq_s if kind == 'q' else bk_s
                    isl = slice(i * 128, (i + 1) * 128)
                    pq = ps.tile([128, 512], F32, tag="ps")
                    for kop in range(NKO // 2):
                        ksl = slice(2 * kop, 2 * kop + 2)
                        nc.tensor.matmul(pq[:], wh[:, ksl, isl],
                                         xnT_hi[ch][:, ksl, :], perf_mode=DR,
                                         start=(kop == 0), stop=False)
                        nc.tensor.matmul(pq[:], wl[:, ksl, isl],
                                         xnT_hi[ch][:, ksl, :], perf_mode=DR,
                                         start=False, stop=False)
                        nc.tensor.matmul(pq[:], wh[:, ksl, isl],
                                         xnT_lo[ch][:, ksl, :], perf_mode=DR,
                                         start=False, stop=(kop == NKO // 2 - 1))
                    if on_act:
                        nc.scalar.activation(
                            out=dst[:], in_=pq[:],
                            func=mybir.ActivationFunctionType.Identity,
                            bias=bias[:, pair:pair + 1], scale=1.0 / WSCALE)
                    else:
                        nc.vector.tensor_scalar(
                            out=dst[:], in0=pq[:],
                            scalar1=1.0 / WSCALE, scalar2=bias[:, pair:pair + 1],
                            op0=mybir.AluOpType.mult,
                            op1=mybir.AluOpType.add)

                def v_tile(g, to):
                    wvt_h, wvt_l = wv_tiles[g]
                    vg = vg_tiles[g]
                    pv = ps.tile([128, 512], F32, tag="ps")
                    tsl = slice((to % 4) * 128, (to % 4 + 1) * 128)
                    for kop in range(NKO // 2):
                        ksl = slice(2 * kop, 2 * kop + 2)
                        nc.tensor.matmul(pv[0:128, 0:256],
                                         xnT_hi[to // 4][:, ksl, tsl],
                                         wvt_h[:, ksl, :], perf_mode=DR,
                                         start=(kop == 0), stop=False)
                        nc.tensor.matmul(pv[0:128, 0:256],
                                         xnT_hi[to // 4][:, ksl, tsl],
                                         wvt_l[:, ksl, :], perf_mode=DR,
                                         start=False, stop=False)
                        nc.tensor.matmul(pv[0:128, 0:256],
                                         xnT_lo[to // 4][:, ksl, tsl],
                                         wvt_h[:, ksl, :], perf_mode=DR,
                                         start=False, stop=(kop == NKO // 2 - 1))
                    nc.vector.scalar_tensor_tensor(
                        out=vg[:, to, :, 0:DH],
                        in0=pv[:, 0:256].rearrange("p (h d) -> p h d", d=DH),
                        scalar=1.0 / WSCALE,
                        in1=bv_r[:, g * 256:(g + 1) * 256].rearrange(
                            "p (h d) -> p h d", d=DH),
                        op0=mybir.AluOpType.mult,
                        op1=mybir.AluOpType.add)

                def head_block(b):
                    """Emit all pair-0..3 QKV units that only need xnT block b.
                    Q/K evacuations ride ACT here (DVE is stage-A-loaded)."""
                    if b < 2:
                        for pair in range(4):
                            qk_group(pair, 'q', b, on_act=True)
                            qk_group(pair, 'k', b, on_act=True)
                    else:
                        for pair in range(4):
                            qk_group(pair, 'k', b, on_act=True)
                    for g in range(2):
                        for to in range(4 * b, 4 * b + 4):
                            v_tile(g, to)

                # ---- Stage A: LN1 (z-norm only) + transpose -> xnT hi/lo,
                # with pairs 0-3 QKV streaming in as blocks complete ----
                with tc.tile_pool(name="workA", bufs=4) as workA, \
                     tc.tile_pool(name="pstA", bufs=2, space="PSUM") as pstA:
                    def finishA(t, x_t, mv, rstd):
                        xn_r = workA.tile([128, C], F32R, tag="xn_r")
                        _ln_stats_b(nc, mv, rstd, x_t[:], xn_r[:], pool=True)
                        for cg in range(2):
                            pt = pstA.tile([128, 4, 128], F32R, tag="pstA")
                            for i in range(4):
                                nc.tensor.transpose(
                                    pt[:, i, :],
                                    xn_r[:, (4 * cg + i) * 128:(4 * cg + i + 1) * 128],
                                    ident_r[:])
                            xsl = (slice(4 * cg, 4 * cg + 4),
                                   slice((t % 4) * 128, (t % 4 + 1) * 128))
                            nc.scalar.activation(
                                out=xnT_hi[t // 4][:, xsl[0], xsl[1]],
                                in_=pt[:],
                                func=mybir.ActivationFunctionType.Copy,
                                bias=0.0, scale=1.0)
                            nc.vector.scalar_tensor_tensor(
                                out=xnT_lo[t // 4][:, xsl[0], xsl[1]],
                                in0=pt[:], scalar=1.0,
                                in1=xnT_hi[t // 4][:, xsl[0], xsl[1]],
                                op0=mybir.AluOpType.mult,
                                op1=mybir.AluOpType.subtract)

                    for pair in range(4):
                        alloc_qk(pair)
                    prevA = None
                    for t in range(T // 128):
                        x_t = workA.tile([128, C], F32, tag="x_t")
                        nc.sync.dma_start(out=x_t[:], in_=x_d[t * 128:(t + 1) * 128, :])
                        if t == 0:
                            load_biases()
                        elif t == 1:
                            load_qk_w(0)
                        elif t == 2:
                            load_v_w(0)
                        elif t == 3:
                            load_qk_w(1)
                            load_v_w(1)
                        mv, rstd = _ln_stats_a(nc, stats, x_t[:], eps_t)
                        if prevA is not None:
                            finishA(*prevA)
                            if prevA[0] % 4 == 3:
                                head_block(prevA[0] // 4)
                        prevA = (t, x_t, mv, rstd)
                    finishA(*prevA)
                    head_block(3)

                # ---- Stages B+C interleaved ----
                prb = abc.enter_context(tc.tile_pool(name="probs", bufs=12))
                onp = abc.enter_context(tc.tile_pool(name="onp", bufs=2))
                otp = abc.enter_context(tc.tile_pool(name="otp", bufs=2))
                pvp = abc.enter_context(tc.tile_pool(name="pvp", bufs=1, space="PSUM"))
                pso = abc.enter_context(tc.tile_pool(name="pso", bufs=1, space="PSUM"))
                ps2 = abc.enter_context(tc.tile_pool(name="ps2", bufs=4, space="PSUM"))
                asm = abc.enter_context(tc.tile_pool(name="att_sm", bufs=3))
                schp = abc.enter_context(tc.tile_pool(name="schp", bufs=3))

                oT_hi = [otp.tile([128, NKO, 512], E4, tag="oThi",
                                  name=f"oThi{i}") for i in range(2)]
                oT_lo = [otp.tile([128, NKO, 512], E5, tag="oTlo",
                                  name=f"oTlo{i}") for i in range(2)]

                def qkv_gen(g):
                    """Q/K for pairs 2g, 2g+1. Yields after each psum group."""
                    load_qk_w(g)
                    for i in range(2):
                        pair = 2 * g + i
                        alloc_qk(pair)
                        for ch in range(TQ // 512):
                            qk_group(pair, 'q', ch)
                            yield
                        for ch in range(TS // 512):
                            qk_group(pair, 'k', ch)
                            yield

                def v_gen(g):
                    """V for heads 4g..4g+3 -> vg tile [128, 16, 4, 65] bf16."""
                    load_v_w(g)
                    for to in range(TS // 128):
                        v_tile(g, to)
                        yield

                exp_cnt = [0]

                def emit_scores_exp(pair, h2, qch):
                    """Scores + exp for one (head, qch) unit. Yields per ktg.

                    Exp routing: ~30% of tiles go Schraudolph (DVE int
                    mul-add, Pool max/clamp) to keep ACT off the critical
                    path; the rest use ACT's native Exp."""
                    qps, kps = qk_tiles[pair]
                    base = h2 * 64
                    pbt = [prb.tile([128, 2, 512], BF16, tag="probsT",
                                    name=f"pb{kg}")
                           for kg in range(TS // 256)]
                    for ktg in range(TS // 256):
                        psc = [ps2.tile([128, 512], F32, tag="psc",
                                        name=f"psc{ktg}j{j}") for j in range(2)]
                        for j in range(2):
                            kt = 2 * ktg + j
                            nc.tensor.matmul(
                                psc[j][:],
                                kps[kt // 4][base:base + DH,
                                             (kt % 4) * 128:(kt % 4 + 1) * 128],
                                qps[qch][base:base + DH, :],
                                start=True, stop=True)
                        exp_cnt[0] += 1
                        sch_frac = 4
                        if (exp_cnt[0] * 3) % 10 < sch_frac:
                            # Schraudolph exp2: bits = y*K1+K2 (DVE),
                            # bitcast to f32, clamp negatives to 0 (Pool)
                            for j in range(2):
                                sch = schp.tile([128, 512],
                                                mybir.dt.int32, tag="sch")
                                nc.vector.tensor_scalar(
                                    out=sch[:], in0=psc[j][:],
                                    scalar1=96817625.34,
                                    scalar2=-484236300.5,
                                    op0=mybir.AluOpType.mult,
                                    op1=mybir.AluOpType.add)
                                nc.gpsimd.tensor_scalar(
                                    out=pbt[ktg][:, j, :],
                                    in0=sch[:].bitcast(F32), scalar1=0.0,
                                    scalar2=None, op0=mybir.AluOpType.max)
                        else:
                            for j in range(2):
                                nc.scalar.activation(
                                    out=pbt[ktg][:, j, :], in_=psc[j][:],
                                    func=mybir.ActivationFunctionType.Exp,
                                    scale=8.0, bias=ebias[:])
                        yield
                    yield ("unit", pair, h2, qch, pbt)

                def emit_pv_norm(pair, h2, qch, pbt, o_norm):
                    """PV + softmax-normalize for a unit whose probs are done."""
                    vg = vg_tiles[pair // 2]
                    hl = (pair * 2 + h2) % 4
                    pvt = pvp.tile([128, 4, DH + 1], F32, tag="pvt")
                    for qt in range(4):
                        for kt in range(TS // 128):
                            nc.tensor.matmul(
                                pvt[:, qt, :],
                                pbt[kt // 2][:, kt % 2,
                                             qt * 128:(qt + 1) * 128],
                                vg[:, kt, hl, :],
                                start=(kt == 0), stop=(kt == TS // 128 - 1))
                    rec = asm.tile([128, 4], F32, tag="rec")
                    nc.vector.reciprocal(out=rec[:], in_=pvt[:, :, DH])
                    nc.vector.tensor_tensor(
                        out=o_norm[:, qch * 4:qch * 4 + 4, h2, :],
                        in0=pvt[:, :, 0:DH], in1=_bcast0(rec[:], DH),
                        op=mybir.AluOpType.mult)

                def emit_oT(pair, o_norm):
                    """Transpose pair's o chunk -> oT hi/lo (c-chunk = pair)."""
                    for ch in range(2):
                        pt = pso.tile([128, 512], F32, tag="pso")
                        for i in range(4):
                            qt = 4 * ch + i
                            nc.tensor.matmul(
                                pt[:, i * 128:(i + 1) * 128],
                                o_norm[:, qt, :, :].rearrange("p h d -> p (h d)"),
                                ident_b[:], start=True, stop=True)
                        nc.vector.tensor_copy(out=oT_hi[ch][:, pair, :], in_=pt[:])
                        nc.vector.scalar_tensor_tensor(
                            out=oT_lo[ch][:, pair, :],
                            in0=pt[:], scalar=1.0,
                            in1=oT_hi[ch][:, pair, :],
                            op0=mybir.AluOpType.mult,
                            op1=mybir.AluOpType.subtract)

                def drain(gen, n=None):
                    k = 0
                    for _ in gen:
                        k += 1
                        if n is not None and k >= n:
                            return True
                    return False

                def gen_chain(g):
                    yield from qkv_gen(g)
                    yield from v_gen(g)

                cur = [None]
                nqk = [2]
                done_chain = [1]  # chains 0,1 fully emitted during stage A

                def pull_qk(pair, n):
                    for _ in range(n):
                        if cur[0] is None and nqk[0] < 4 and nqk[0] <= pair // 2 + 1:
                            cur[0] = gen_chain(nqk[0])
                            nqk[0] += 1
                        if cur[0] is None:
                            return
                        if not drain(cur[0], 1):
                            done_chain[0] = nqk[0] - 1
                            cur[0] = None

                ycnt = [0]
                o_norms = {}
                pending = [None]  # (pair, h2, qch, pbt)

                def flush_pending():
                    if pending[0] is not None:
                        p_, h2_, qch_, pbt_ = pending[0]
                        # PV contracts over every vg[p_//2] tile: the whole
                        # chain must be emitted before the PV matmuls
                        while done_chain[0] < p_ // 2:
                            pull_qk(p_, 1)
                        emit_pv_norm(p_, h2_, qch_, pbt_, o_norms[p_])
                        pending[0] = None
                        if h2_ == 1 and qch_ == TQ // 512 - 1:
                            emit_oT(p_, o_norms.pop(p_))

                for pair in range(H // 2):
                    while pair not in qk_tiles or pair // 2 not in vg_tiles:
                        pull_qk(pair, 1)
                    o_norms[pair] = onp.tile([128, TQ // 128, 2, DH], BF16,
                                             tag="o_norm", name=f"o_norm{pair}")
                    for h2 in range(2):
                        for qch in range(TQ // 512):
                            for tok in emit_scores_exp(pair, h2, qch):
                                if isinstance(tok, tuple):
                                    flush_pending()
                                    pending[0] = (pair, h2, qch, tok[4])
                                else:
                                    ycnt[0] += 1
                                    if ycnt[0] % (3 if pair < 4 else 2) == 0:
                                        pull_qk(pair, 1)
                flush_pending()

            # ============ Stage D: oT split, Wo (3-term fp8), residual, LN2 ====
            with contextlib.ExitStack() as dstk:
                x2p = dstk.enter_context(tc.tile_pool(name="x2p", bufs=1))
                xn2p = dstk.enter_context(tc.tile_pool(name="xn2p", bufs=2))
                x2 = x2p.tile([128, TQ // 128, C], BF16, tag="x2")
                xn2_hi = [xn2p.tile([128, NKO, 512], E4, tag="xn2hi",
                                    name=f"xn2hi{i}") for i in range(2)]
                xn2_lo = [xn2p.tile([128, NKO, 512], E5, tag="xn2lo",
                                    name=f"xn2lo{i}") for i in range(2)]

                b2rp = dstk.enter_context(tc.tile_pool(name="b2rp", bufs=1))
                b2r = b2rp.tile([128, C], F32, tag="b2r")
                nc.sync.dma_start(out=b2r[:], in_=b2r_d[:, :])
                pst2 = dstk.enter_context(tc.tile_pool(name="pst2", bufs=3,
                                                       space="PSUM"))
                psE = dstk.enter_context(tc.tile_pool(name="psE", bufs=3,
                                                      space="PSUM"))
                with contextlib.ExitStack() as dd:
                    wop = dd.enter_context(tc.tile_pool(name="wop", bufs=1))
                    workD = dd.enter_context(tc.tile_pool(name="workD", bufs=4))
                    wo_hi = wop.tile([128, NKO, C], E4, tag="wohi")
                    wo_lo = wop.tile([128, NKO, C], E5, tag="wolo")
                    nc.sync.dma_start(out=wo_hi[:],
                                      in_=woh_d.rearrange("(o p) f -> p o f", p=128))
                    nc.sync.dma_start(out=wo_lo[:],
                                      in_=wol_d.rearrange("(o p) f -> p o f", p=128))

                    # aoT back to token-major + residual -> x2; LN2 -> xn2 hi/lo
                    def finishD(t, mv, rstd):
                        xn2_r = workD.tile([128, C], F32R, tag="xn2_r")
                        _ln_stats_b(nc, mv, rstd, x2[:, t, :], xn2_r[:],
                                    pool=False)
                        for cg in range(2):
                            pt = pst2.tile([128, 4, 128], F32R, tag="pst2")
                            for i in range(4):
                                c = 4 * cg + i
                                nc.tensor.transpose(
                                    pt[:, i, :],
                                    xn2_r[:, c * 128:(c + 1) * 128], ident_r[:])
                            xsl = (slice(4 * cg, 4 * cg + 4),
                                   slice((t % 4) * 128, (t % 4 + 1) * 128))
                            nc.scalar.activation(
                                out=xn2_hi[t // 4][:, xsl[0], xsl[1]],
                                in_=pt[:], func=mybir.ActivationFunctionType.Copy,
                                bias=0.0, scale=1.0)
                            nc.vector.scalar_tensor_tensor(
                                out=xn2_lo[t // 4][:, xsl[0], xsl[1]],
                                in0=pt[:], scalar=1.0,
                                in1=xn2_hi[t // 4][:, xsl[0], xsl[1]],
                                op0=mybir.AluOpType.mult,
                                op1=mybir.AluOpType.subtract)

                    prevD = None
                    for t in range(TQ // 128):
                        x_t = workD.tile([128, C], F32, tag="x_t")
                        nc.sync.dma_start(out=x_t[:],
                                          in_=xpb_d[t * 128:(t + 1) * 128, :])
                        tsl = slice((t % 4) * 128, (t % 4 + 1) * 128)
                        for half in range(2):
                            hsl = slice(half * 512, (half + 1) * 512)
                            pw = psE.tile([128, 512], F32, tag="psE")
                            for kop in range(NKO // 2):
                                ksl = slice(2 * kop, 2 * kop + 2)
                                nc.tensor.matmul(pw[:], oT_hi[t // 4][:, ksl, tsl],
                                                 wo_hi[:, ksl, hsl], perf_mode=DR,
                                                 start=(kop == 0), stop=False)
                                nc.tensor.matmul(pw[:], oT_hi[t // 4][:, ksl, tsl],
                                                 wo_lo[:, ksl, hsl], perf_mode=DR,
                                                 start=False, stop=False)
                                nc.tensor.matmul(pw[:], oT_lo[t // 4][:, ksl, tsl],
                                                 wo_hi[:, ksl, hsl], perf_mode=DR,
                                                 start=False, stop=(kop == NKO // 2 - 1))
                            nc.vector.scalar_tensor_tensor(
                                out=x2[:, t, hsl], in0=pw[:],
                                scalar=1.0 / WSCALE, in1=x_t[:, hsl],
                                op0=mybir.AluOpType.mult,
                                op1=mybir.AluOpType.add)
                        mv, rstd = _ln_stats_a(nc, stats, x2[:, t, :], eps_t)
                        if prevD is not None:
                            finishD(*prevD)
                        prevD = (t, mv, rstd)
                    finishD(*prevD)

                # ============ Stage E: FFN up (W1, relu) 3-term fp8 ============
                h1p = dstk.enter_context(tc.tile_pool(name="h1p", bufs=1))
                h1_hi = h1p.tile([128, DFF // 128, TQ], E4, tag="h1hi")
                h1_lo = h1p.tile([128, DFF // 128, TQ], E5, tag="h1lo")
                w1h_r = w1h_d.rearrange("(o p) f -> p o f", p=128)
                w1l_r = w1l_d.rearrange("(o p) f -> p o f", p=128)
                w2fp = dstk.enter_context(tc.tile_pool(name="w2fp", bufs=1))
                w2fh = w2fp.tile([128, DFF // 128, C], E4, tag="w2fh")
                w2fl = w2fp.tile([128, DFF // 128, C], E5, tag="w2fl")
                w2h_r = w2h_d.rearrange("(o p) f -> p o f", p=128)
                w2l_r = w2l_d.rearrange("(o p) f -> p o f", p=128)
                with tc.tile_pool(name="w1p", bufs=2) as w1p:
                    for blk in range(DFF // 512):
                        w1th = w1p.tile([128, NKO, 512], E4, tag="w1th")
                        w1tl = w1p.tile([128, NKO, 512], E5, tag="w1tl")
                        nc.sync.dma_start(out=w1th[:],
                                          in_=w1h_r[:, :, blk * 512:(blk + 1) * 512])
                        nc.sync.dma_start(out=w1tl[:],
                                          in_=w1l_r[:, :, blk * 512:(blk + 1) * 512])
                        # stream the full-width W2 tiles in behind the W1
                        # blocks (the DMA lane has slack during stage E)
                        if blk >= 4:
                            c4 = (blk - 4) * 256
                            nc.sync.dma_start(out=w2fh[:, :, c4:c4 + 256],
                                              in_=w2h_r[:, :, c4:c4 + 256])
                            nc.sync.dma_start(out=w2fl[:, :, c4:c4 + 256],
                                              in_=w2l_r[:, :, c4:c4 + 256])
                        for ch in range(TQ // 512):
                            csl = slice(ch * 512, (ch + 1) * 512)
                            for fs in range(4):
                                f = blk * 4 + fs
                                fsl = slice(fs * 128, (fs + 1) * 128)
                                ph = psE.tile([128, 512], F32, tag="psE")
                                for kop in range(NKO // 2):
                                    ksl = slice(2 * kop, 2 * kop + 2)
                                    nc.tensor.matmul(ph[:], w1th[:, ksl, fsl],
                                                     xn2_hi[ch][:, ksl, :], perf_mode=DR,
                                                     start=(kop == 0), stop=False)
                                    nc.tensor.matmul(ph[:], w1tl[:, ksl, fsl],
                                                     xn2_hi[ch][:, ksl, :], perf_mode=DR,
                                                     start=False, stop=False)
                                    nc.tensor.matmul(ph[:], w1th[:, ksl, fsl],
                                                     xn2_lo[ch][:, ksl, :], perf_mode=DR,
                                                     start=False,
                                                     stop=(kop == NKO // 2 - 1))
                                nc.scalar.activation(
                                    out=h1_hi[:, f, csl], in_=ph[:],
                                    func=mybir.ActivationFunctionType.Relu,
                                    bias=b1_s[:, f:f + 1], scale=1.0)
                                nc.vector.scalar_tensor_tensor(
                                    out=h1_lo[:, f, csl], in0=ph[:], scalar=0.0,
                                    in1=h1_hi[:, f, csl],
                                    op0=mybir.AluOpType.max,
                                    op1=mybir.AluOpType.subtract)

                # ============ Stage F: FFN down (W2) 3-term fp8 + residual ======
                # Token-major output: out[t, c] = h1^T @ W2 — lhsT is the
                # (already feature-major) h1, so no final transposes and the
                # residual + b2 ride the DVE evacuation directly.
                with tc.tile_pool(name="workF", bufs=3) as workF:
                    for t in range(TQ // 128):
                        out_t = workF.tile([128, C], F32, tag="out_t")
                        tsl = slice(t * 128, (t + 1) * 128)
                        for half in range(2):
                            hsl = slice(half * 512, (half + 1) * 512)
                            po2 = psE.tile([128, 512], F32, tag="psE")
                            for kop in range(DFF // 256):
                                ksl = slice(2 * kop, 2 * kop + 2)
                                nc.tensor.matmul(po2[:], h1_hi[:, ksl, tsl],
                                                 w2fh[:, ksl, hsl], perf_mode=DR,
                                                 start=(kop == 0), stop=False)
                                nc.tensor.matmul(po2[:], h1_hi[:, ksl, tsl],
                                                 w2fl[:, ksl, hsl], perf_mode=DR,
                                                 start=False, stop=False)
                                nc.tensor.matmul(po2[:], h1_lo[:, ksl, tsl],
                                                 w2fh[:, ksl, hsl], perf_mode=DR,
                                                 start=False,
                                                 stop=(kop == DFF // 256 - 1))
                            nc.vector.scalar_tensor_tensor(
                                out=out_t[:, hsl], in0=po2[:],
                                scalar=1.0 / (WSCALE * WSCALE),
                                in1=b2r[:, hsl],
                                op0=mybir.AluOpType.mult,
                                op1=mybir.AluOpType.add)
                            nc.vector.tensor_tensor(
                                out=out_t[:, hsl], in0=out_t[:, hsl],
                                in1=x2[:, t, hsl],
                                op=mybir.AluOpType.add)
                            nc.sync.dma_start(
                                out=out_d[t * 128:(t + 1) * 128, hsl],
                                in_=out_t[:, hsl])

    nc.finalize()
    _legalize_sem_waits(nc)
    return nc


_NC_CACHE = None


def _get_nc():
    global _NC_CACHE
    if _NC_CACHE is None:
        _NC_CACHE = _build_nc()
    return _NC_CACHE


def _split_w(w, scale=WSCALE):
    ws = np.asarray(w, np.float32) * scale
    hi = ws.astype(ml_dtypes.float8_e4m3)
    lo = (ws - hi.astype(np.float32)).astype(ml_dtypes.float8_e5m2)
    return np.ascontiguousarray(hi), np.ascontiguousarray(lo)


def _shard_inputs(inputs):
    x = np.asarray(inputs["x"], np.float32)
    ln1_g = np.asarray(inputs["ln1_g"], np.float32).reshape(C)
    ln1_b = np.asarray(inputs["ln1_b"], np.float32).reshape(C)
    ln2_g = np.asarray(inputs["ln2_g"], np.float32).reshape(C)
    ln2_b = np.asarray(inputs["ln2_b"], np.float32).reshape(C)
    wq = np.ascontiguousarray(
        np.transpose(np.asarray(inputs["Wq"], np.float32), (1, 0, 2)).reshape(C, C))
    wk = np.ascontiguousarray(
        np.transpose(np.asarray(inputs["Wk"], np.float32), (1, 0, 2)).reshape(C, C))
    wv = np.ascontiguousarray(
        np.transpose(np.asarray(inputs["Wv"], np.float32), (1, 0, 2)).reshape(C, C))
    wo = np.asarray(inputs["Wo"], np.float32)
    w1 = np.asarray(inputs["W1"], np.float32)
    w2 = np.asarray(inputs["W2"], np.float32)

    # fold LN affine into the consuming weights/biases
    bq = np.asarray(inputs["bq"], np.float32).reshape(C) + ln1_b @ wq
    bk = np.asarray(inputs["bk"], np.float32).reshape(C) + ln1_b @ wk
    bv = np.asarray(inputs["bv"], np.float32).reshape(C) + ln1_b @ wv
    wq = np.ascontiguousarray(ln1_g[:, None] * wq)
    wk = np.ascontiguousarray(ln1_g[:, None] * wk)
    wv = np.ascontiguousarray(ln1_g[:, None] * wv)
    b1 = WSCALE * (np.asarray(inputs["b1"], np.float32).reshape(DFF) + ln2_b @ w1)
    assert np.abs(b1).max() == 0.0, "nonzero effective W1 bias unsupported by lo-split"
    w1g = ln2_g[:, None] * w1

    wqh, wql = _split_w(wq)
    wkh, wkl = _split_w(wk)
    wvh, wvl = _split_w(wv)
    woh, wol = _split_w(wo)
    w1h, w1l = _split_w(w1g)
    w2h, w2l = _split_w(w2)

    shared = {
        "wqh": wqh, "wql": wql, "wkh": wkh, "wkl": wkl, "wvh": wvh, "wvl": wvl,
        "woh": woh, "wol": wol, "w1h": w1h, "w1l": w1l, "w2h": w2h, "w2l": w2l,
        "bq": bq, "bk": bk,
        "bv": np.ascontiguousarray(
            np.broadcast_to(bv.astype(ml_dtypes.bfloat16), (128, C))),
        "bo": np.asarray(inputs["bo"], np.float32).reshape(C),
        "b1": b1,
        "b2": np.asarray(inputs["b2"], np.float32).reshape(C),
        "b2r": np.ascontiguousarray(np.broadcast_to(
            np.asarray(inputs["b2"], np.float32).reshape(1, C), (128, C))),
    }
    bo_v = np.asarray(inputs["bo"], np.float32).reshape(C)
    in_maps = []
    for c in range(N_CORES):
        b, half = c // 2, c % 2
        own = x[b, half * TQ:(half + 1) * TQ]
        other = x[b, (1 - half) * TQ:(2 - half) * TQ]
        x_perm = np.ascontiguousarray(np.concatenate([own, other], axis=0))
        xpb = np.ascontiguousarray(own + bo_v)
        in_maps.append(dict(shared, x=x_perm, xpb=xpb))
    return in_maps


def _run(inputs, **spmd_kwargs):
    nc = _get_nc()
    in_maps = _shard_inputs(inputs)
    res = run_bass_kernel_spmd(nc, in_maps, core_ids=list(range(N_CORES)), **spmd_kwargs)
    out = np.empty((B, T, C), np.float32)
    for c in range(N_CORES):
        b, half = c // 2, c % 2
        out[b, half * TQ:(half + 1) * TQ] = res.results[c]["out"]
    return out, res


def kernel(**inputs) -> np.ndarray:
    out, _ = _run(inputs)
    return out



# revision 68
# speedup vs baseline: 1.0011x; 1.0011x over previous
"""Trainium2 Bass kernel for a pre-norm transformer block (MHSA + FFN).

Sharding: 8 cores, data parallel over (batch, seq-half). Core c handles
batch c//2, sequence half c%2. Inputs are permuted so each core's own
1024 tokens come first; attention K/V run over all 2048 tokens of the
batch (softmax is permutation invariant).

Numerics: ALL five projections (Q/K/V/Wo/W1/W2) run as 3-term
compensated fp8 DoubleRow (hi=e4m3, lo=e5m2, x@W ~= xh@Wh + xh@Wl +
xl@Wh, 4x bf16 matmul throughput per term), with weights pre-scaled x32
and split host-side; the normalized activations are split on-chip after
the TensorE transpose. Scores stay f32r (softmax logits are ~N(0,26^2)
— direct fp8 there flips argmaxes); softmax probs in bf16 with a
constant exp shift; PV runs probs-stationary so only the 65-wide
(dh+denominator) V operand streams. LayerNorm gains/biases are folded
into the downstream weights/biases host-side, so on-chip LN is pure
z-normalization.

Scheduling: the softmax exp (the largest non-matmul cost, ~218us if
ACT-only) is split ~60/40 between ACT's native Exp and a Schraudolph
exp2 (DVE int mul-add + Pool max/clamp; GPSIMD cannot read PSUM, so
the PSUM-side op must be the DVE one). Stage A (LN1+transpose+split)
streams pairs 0-3's Q/K/V matmuls in as each transposed 512-token
block completes, so the PE is busy during LN; DMA issue order is
arranged so the first x tiles are not stuck behind weight/bias loads
on the serial DMA lane.

Wo and W2 both run token-major (lhsT = the already-feature-major oT /
h1), so the attention and FFN outputs land directly in token order:
no aoT/ffnT intermediates, no transpose-back matmuls, and the
residual + bias ride the single DVE evacuation of each PSUM group
(bo is pre-added to the x residual host-side, b2 comes in replicated
across partitions). x2 is kept in bf16 to fit the full-width W2 tiles
in SBUF. PV safety: a chain-completion tracker drains each QKV
generator fully before the first PV that contracts over its V tiles.
"""
import contextlib

import numpy as np
import ml_dtypes

import concourse.bass as bass
import concourse.tile as tile
import concourse.mybir as mybir
from concourse.bass_utils import run_bass_kernel_spmd
from concourse.masks import make_identity

B, T, C = 4, 2048, 1024
H, DH = 16, 64
DFF = 4 * C
N_CORES = 8
TQ = T // 2          # tokens owned per core
TS = T               # key/value tokens per core
NKO = C // 128       # 8 contraction tiles for C
F32R = mybir.dt.float32r
F32 = mybir.dt.float32
BF16 = mybir.dt.bfloat16
E4 = mybir.dt.float8e4
E5 = mybir.dt.float8e5
EXP_BIAS = -128.0
EPS = 1e-5
WSCALE = 32.0
DR = mybir.MatmulPerfMode.DoubleRow

# ---------------------------------------------------------------------------
# Compat: this walrus build accepts at most 1 sem-wait per regular
# instruction (2 per InstEventSemaphore). bacc misses some tile-generated
# instructions, so split waits ourselves after finalize.
_ev_counter = [0]


def _legalize_sem_waits(nc):
    for func in nc.m.functions:
        for bb in func.blocks:
            new = []
            changed = False
            for inst in bb.instructions:
                si = inst.sync_info
                cap = 2 if isinstance(inst, mybir.InstEventSemaphore) else 1
                if si is not None and len(si.on_wait) > cap:
                    waits = list(si.on_wait)
                    for i in range(cap, len(waits), 2):
                        _ev_counter[0] += 1
                        e = mybir.InstEventSemaphore(
                            name=f"EVSPLIT-{_ev_counter[0]}", ins=[], outs=[])
                        e.engine = inst.engine
                        e.sync_info = mybir.SyncInfo(
                            on_wait=waits[i:i + 2], on_update=[])
                        new.append(e)
                    inst.sync_info = mybir.SyncInfo(
                        on_wait=waits[:cap], on_update=list(si.on_update))
                    changed = True
                new.append(inst)
            if changed:
                bb.instructions = new


# ---------------------------------------------------------------------------

def _ln_stats_a(nc, stats, x_ap, eps_t):
    """bn stats + sqrt(var+eps) for x_ap [128, C]; returns (mv, rstd)."""
    st = stats.tile([128, 2, 6], F32, tag="bnstats")
    mv = stats.tile([128, 2], F32, tag="bnaggr")
    xg = x_ap.rearrange("p (s d) -> p s d", s=2)
    for s in range(2):
        nc.vector.bn_stats(out=st[:, s, :], in_=xg[:, s, :])
    nc.vector.bn_aggr(out=mv[:], in_=st[:])
    rstd = stats.tile([128, 1], F32, tag="rstd")
    nc.scalar.activation(out=rstd[:], in_=mv[:, 1:2],
                         func=mybir.ActivationFunctionType.Sqrt,
                         bias=eps_t[:], scale=1.0)
    return mv, rstd


def _ln_stats_b(nc, mv, rstd, x_ap, out_ap, pool=False):
    """finish z-normalize: recip + (x - mu) * rstd."""
    nc.vector.reciprocal(out=rstd[:], in_=rstd[:])
    eng = nc.gpsimd if pool else nc.vector
    eng.tensor_scalar(out=out_ap, in0=x_ap,
                      scalar1=mv[:, 0:1], scalar2=rstd[:],
                      op0=mybir.AluOpType.subtract,
                      op1=mybir.AluOpType.mult)


def _bcast0(ap, free):
    """Broadcast a [128, n] AP along a new stride-0 free dim of size `free`."""
    return bass.AP(tensor=ap.tensor, offset=ap.offset,
                   ap=[list(d) for d in ap.ap] + [[0, free]])


def _build_nc():
    nc = bass.Bass()

    # ---- I/O ----
    x_d = nc.dram_tensor("x", [T, C], F32, kind="ExternalInput")
    xpb_d = nc.dram_tensor("xpb", [TQ, C], BF16, kind="ExternalInput")
    wqh_d = nc.dram_tensor("wqh", [C, C], E4, kind="ExternalInput")
    wql_d = nc.dram_tensor("wql", [C, C], E5, kind="ExternalInput")
    wkh_d = nc.dram_tensor("wkh", [C, C], E4, kind="ExternalInput")
    wkl_d = nc.dram_tensor("wkl", [C, C], E5, kind="ExternalInput")
    wvh_d = nc.dram_tensor("wvh", [C, C], E4, kind="ExternalInput")
    wvl_d = nc.dram_tensor("wvl", [C, C], E5, kind="ExternalInput")
    woh_d = nc.dram_tensor("woh", [C, C], E4, kind="ExternalInput")
    wol_d = nc.dram_tensor("wol", [C, C], E5, kind="ExternalInput")
    w1h_d = nc.dram_tensor("w1h", [C, DFF], E4, kind="ExternalInput")
    w1l_d = nc.dram_tensor("w1l", [C, DFF], E5, kind="ExternalInput")
    w2h_d = nc.dram_tensor("w2h", [DFF, C], E4, kind="ExternalInput")
    w2l_d = nc.dram_tensor("w2l", [DFF, C], E5, kind="ExternalInput")
    bq_d = nc.dram_tensor("bq", [C], F32, kind="ExternalInput")
    bk_d = nc.dram_tensor("bk", [C], F32, kind="ExternalInput")
    bv_d = nc.dram_tensor("bv", [128, C], BF16, kind="ExternalInput")
    bo_d = nc.dram_tensor("bo", [C], F32, kind="ExternalInput")
    b1_d = nc.dram_tensor("b1", [DFF], F32, kind="ExternalInput")
    b2_d = nc.dram_tensor("b2", [C], F32, kind="ExternalInput")
    b2r_d = nc.dram_tensor("b2r", [128, C], F32, kind="ExternalInput")
    out_d = nc.dram_tensor("out", [TQ, C], BF16, kind="ExternalOutput")

    wqh_r = wqh_d.rearrange("(o p) f -> p o f", p=128)
    wql_r = wql_d.rearrange("(o p) f -> p o f", p=128)
    wkh_r = wkh_d.rearrange("(o p) f -> p o f", p=128)
    wkl_r = wkl_d.rearrange("(o p) f -> p o f", p=128)
    wvh_r = wvh_d.rearrange("(o p) f -> p o f", p=128)
    wvl_r = wvl_d.rearrange("(o p) f -> p o f", p=128)

    with tile.TileContext(nc) as tc:
        with contextlib.ExitStack() as top:
            consts = top.enter_context(tc.tile_pool(name="consts", bufs=1))
            ps = top.enter_context(tc.tile_pool(name="ps", bufs=2, space="PSUM"))
            stats = top.enter_context(tc.tile_pool(name="stats", bufs=8))

            ident_b = consts.tile([128, 128], BF16, tag="identb")
            make_identity(nc, ident_b)
            ident_r = consts.tile([128, 128], F32R, tag="identr")
            nc.vector.tensor_copy(out=ident_r[:], in_=ident_b[:])
            ebias = consts.tile([128, 1], F32, tag="ebias")
            nc.vector.memset(ebias[:], EXP_BIAS)
            eps_t = consts.tile([128, 1], F32, tag="eps")
            nc.vector.memset(eps_t[:], EPS)
            bq_s = consts.tile([128, NKO], F32, tag="bq")
            bk_s = consts.tile([128, NKO], F32, tag="bk")
            bo_s = consts.tile([128, NKO], F32, tag="bo")
            b2_s = consts.tile([128, NKO], F32, tag="b2")
            b1_s = consts.tile([128, DFF // 128], F32, tag="b1")
            bv_r = consts.tile([128, C], BF16, tag="bvr")

            def load_biases():
                for dst, src_ in ((bq_s, bq_d), (bk_s, bk_d), (bo_s, bo_d),
                                  (b2_s, b2_d), (b1_s, b1_d)):
                    nc.sync.dma_start(out=dst[:],
                                      in_=src_.rearrange("(o p) -> p o", p=128))
                nc.sync.dma_start(out=bv_r[:], in_=bv_d[:, :])

            # ============ Stages A-C: LN1, QKV, attention ============
            with contextlib.ExitStack() as abc:
                xnp = abc.enter_context(tc.tile_pool(name="xnp", bufs=8))
                xnT_hi = [xnp.tile([128, NKO, 512], E4, tag="xnThi",
                                   name=f"xnThi{i}") for i in range(4)]
                xnT_lo = [xnp.tile([128, NKO, 512], E5, tag="xnTlo",
                                   name=f"xnTlo{i}") for i in range(4)]
                wgp = abc.enter_context(tc.tile_pool(name="wgp", bufs=2))
                qkp = abc.enter_context(tc.tile_pool(name="qkp", bufs=2))
                vgp = abc.enter_context(tc.tile_pool(name="vgp", bufs=2))

                qk_tiles = {}
                vg_tiles = {}
                wq_tiles = {}
                wv_tiles = {}

                def load_qk_w(g):
                    wqt_h = wgp.tile([128, NKO, 256], E4, tag="wqth")
                    wqt_l = wgp.tile([128, NKO, 256], E5, tag="wqtl")
                    wkt_h = wgp.tile([128, NKO, 256], E4, tag="wkth")
                    wkt_l = wgp.tile([128, NKO, 256], E5, tag="wktl")
                    fsl_w = slice(g * 256, (g + 1) * 256)
                    nc.sync.dma_start(out=wqt_h[:], in_=wqh_r[:, :, fsl_w])
                    nc.sync.dma_start(out=wqt_l[:], in_=wql_r[:, :, fsl_w])
                    nc.sync.dma_start(out=wkt_h[:], in_=wkh_r[:, :, fsl_w])
                    nc.sync.dma_start(out=wkt_l[:], in_=wkl_r[:, :, fsl_w])
                    wq_tiles[g] = (wqt_h, wqt_l, wkt_h, wkt_l)

                def load_v_w(g):
                    wvt_h = wgp.tile([128, NKO, 256], E4, tag="wvth")
                    wvt_l = wgp.tile([128, NKO, 256], E5, tag="wvtl")
                    nc.sync.dma_start(out=wvt_h[:], in_=wvh_r[:, :, g * 256:(g + 1) * 256])
                    nc.sync.dma_start(out=wvt_l[:], in_=wvl_r[:, :, g * 256:(g + 1) * 256])
                    wv_tiles[g] = (wvt_h, wvt_l)
                    vg = vgp.tile([128, TS // 128, 4, 65], BF16, tag="vg")
                    vg_tiles[g] = vg
                    nc.vector.memset(vg[:, :, :, DH:DH + 1], 1.0)

                def alloc_qk(pair):
                    i = pair % 2
                    qps = [qkp.tile([128, 512], F32R, tag=f"qp{i}c{ch}",
                                    name=f"qp{pair}c{ch}")
                           for ch in range(TQ // 512)]
                    kps = [qkp.tile([128, 512], F32R, tag=f"kp{i}c{ch}",
                                    name=f"kp{pair}c{ch}")
                           for ch in range(TS // 512)]
                    qk_tiles[pair] = (qps, kps)

                def qk_group(pair, kind, ch, on_act=False):
                    """One Q or K psum group (3-term fp8 DR) + evacuation."""
                    g, i = pair // 2, pair % 2
                    wqt_h, wqt_l, wkt_h, wkt_l = wq_tiles[g]
                    wh, wl = (wqt_h, wqt_l) if kind == 'q' else (wkt_h, wkt_l)
                    dst = qk_tiles[pair][0 if kind == 'q' else 1][ch]
                    bias = bq_s if kind == 'q' else bk_s
                    isl = slice(i * 128, (i + 1) * 128)
                    pq = ps.tile([128, 512], F32, tag="ps")
                    for kop in range(NKO // 2):
                        ksl = slice(2 * kop, 2 * kop + 2)
                        nc.tensor.matmul(pq[:], wh[:, ksl, isl],
                                         xnT_hi[ch][:, ksl, :], perf_mode=DR,
                                         start=(kop == 0), stop=False)
                        nc.tensor.matmul(pq[:], wl[:, ksl, isl],
                                         xnT_hi[ch][:, ksl, :], perf_mode=DR,
                                         start=False, stop=False)
                        nc.tensor.matmul(pq[:], wh[:, ksl, isl],
                                         xnT_lo[ch][:, ksl, :], perf_mode=DR,
                                         start=False, stop=(kop == NKO // 2 - 1))
                    if on_act:
                        nc.scalar.activation(
                            out=dst[:], in_=pq[:],
                            func=mybir.ActivationFunctionType.Identity,
                            bias=bias[:, pair:pair + 1], scale=1.0 / WSCALE)
                    else:
                        nc.vector.tensor_scalar(
                            out=dst[:], in0=pq[:],
                            scalar1=1.0 / WSCALE, scalar2=bias[:, pair:pair + 1],
                            op0=mybir.AluOpType.mult,
                            op1=mybir.AluOpType.add)

                def v_tile(g, to):
                    wvt_h, wvt_l = wv_tiles[g]
                    vg = vg_tiles[g]
                    pv = ps.tile([128, 512], F32, tag="ps")
                    tsl = slice((to % 4) * 128, (to % 4 + 1) * 128)
                    for kop in range(NKO // 2):
                        ksl = slice(2 * kop, 2 * kop + 2)
                        nc.tensor.matmul(pv[0:128, 0:256],
                                         xnT_hi[to // 4][:, ksl, tsl],
                                         wvt_h[:, ksl, :], perf_mode=DR,
                                         start=(kop == 0), stop=False)
                        nc.tensor.matmul(pv[0:128, 0:256],
                                         xnT_hi[to // 4][:, ksl, tsl],
                                         wvt_l[:, ksl, :], perf_mode=DR,
                                         start=False, stop=False)
                        nc.tensor.matmul(pv[0:128, 0:256],
                                         xnT_lo[to // 4][:, ksl, tsl],
                                         wvt_h[:, ksl, :], perf_mode=DR,
                                         start=False, stop=(kop == NKO // 2 - 1))
                    nc.vector.scalar_tensor_tensor(
                        out=vg[:, to, :, 0:DH],
                        in0=pv[:, 0:256].rearrange("p (h d) -> p h d", d=DH),
                        scalar=1.0 / WSCALE,
                        in1=bv_r[:, g * 256:(g + 1) * 256].rearrange(
                            "p (h d) -> p h d", d=DH),
                        op0=mybir.AluOpType.mult,
                        op1=mybir.AluOpType.add)

                def head_block(b):
                    """Emit all pair-0..3 QKV units that only need xnT block b.
                    Q/K evacuations ride ACT here (DVE is stage-A-loaded)."""
                    if b < 2:
                        for pair in range(4):
                            qk_group(pair, 'q', b, on_act=True)
                            qk_group(pair, 'k', b, on_act=True)
                    else:
                        for pair in range(4):
                            qk_group(pair, 'k', b, on_act=True)
                    for g in range(2):
                        for to in range(4 * b, 4 * b + 4):
                            v_tile(g, to)

                # ---- Stage A: LN1 (z-norm only) + transpose -> xnT hi/lo,
                # with pairs 0-3 QKV streaming in as blocks complete ----
                with tc.tile_pool(name="workA", bufs=4) as workA, \
                     tc.tile_pool(name="pstA", bufs=2, space="PSUM") as pstA:
                    def finishA(t, x_t, mv, rstd):
                        xn_r = workA.tile([128, C], F32R, tag="xn_r")
                        _ln_stats_b(nc, mv, rstd, x_t[:], xn_r[:], pool=True)
                        for cg in range(2):
                            pt = pstA.tile([128, 4, 128], F32R, tag="pstA")
                            for i in range(4):
                                nc.tensor.transpose(
                                    pt[:, i, :],
                                    xn_r[:, (4 * cg + i) * 128:(4 * cg + i + 1) * 128],
                                    ident_r[:])
                            xsl = (slice(4 * cg, 4 * cg + 4),
                                   slice((t % 4) * 128, (t % 4 + 1) * 128))
                            nc.scalar.activation(
                                out=xnT_hi[t // 4][:, xsl[0], xsl[1]],
                                in_=pt[:],
                                func=mybir.ActivationFunctionType.Copy,
                                bias=0.0, scale=1.0)
                            nc.vector.scalar_tensor_tensor(
                                out=xnT_lo[t // 4][:, xsl[0], xsl[1]],
                                in0=pt[:], scalar=1.0,
                                in1=xnT_hi[t // 4][:, xsl[0], xsl[1]],
                                op0=mybir.AluOpType.mult,
                                op1=mybir.AluOpType.subtract)

                    for pair in range(4):
                        alloc_qk(pair)
                    prevA = None
                    for t in range(T // 128):
                        x_t = workA.tile([128, C], F32, tag="x_t")
                        nc.sync.dma_start(out=x_t[:], in_=x_d[t * 128:(t + 1) * 128, :])
                        if t == 0:
                            load_biases()
                        elif t == 1:
                            load_qk_w(0)
                        elif t == 2:
                            load_v_w(0)
                        elif t == 3:
                            load_qk_w(1)
                            load_v_w(1)
                        mv, rstd = _ln_stats_a(nc, stats, x_t[:], eps_t)
                        if prevA is not None:
                            finishA(*prevA)
                            if prevA[0] % 4 == 3:
                                head_block(prevA[0] // 4)
                        prevA = (t, x_t, mv, rstd)
                    finishA(*prevA)
                    head_block(3)

                # ---- Stages B+C interleaved ----
                prb = abc.enter_context(tc.tile_pool(name="probs", bufs=12))
                onp = abc.enter_context(tc.tile_pool(name="onp", bufs=2))
                otp = abc.enter_context(tc.tile_pool(name="otp", bufs=2))
                pvp = abc.enter_context(tc.tile_pool(name="pvp", bufs=1, space="PSUM"))
                pso = abc.enter_context(tc.tile_pool(name="pso", bufs=1, space="PSUM"))
                ps2 = abc.enter_context(tc.tile_pool(name="ps2", bufs=4, space="PSUM"))
                asm = abc.enter_context(tc.tile_pool(name="att_sm", bufs=3))
                schp = abc.enter_context(tc.tile_pool(name="schp", bufs=3))

                oT_hi = [otp.tile([128, NKO, 512], E4, tag="oThi",
                                  name=f"oThi{i}") for i in range(2)]
                oT_lo = [otp.tile([128, NKO, 512], E5, tag="oTlo",
                                  name=f"oTlo{i}") for i in range(2)]

                def qkv_gen(g):
                    """Q/K for pairs 2g, 2g+1. Yields after each psum group."""
                    load_qk_w(g)
                    for i in range(2):
                        pair = 2 * g + i
                        alloc_qk(pair)
                        for ch in range(TQ // 512):
                            qk_group(pair, 'q', ch)
                            yield
                        for ch in range(TS // 512):
                            qk_group(pair, 'k', ch)
                            yield

                def v_gen(g):
                    """V for heads 4g..4g+3 -> vg tile [128, 16, 4, 65] bf16."""
                    load_v_w(g)
                    for to in range(TS // 128):
                        v_tile(g, to)
                        yield

                exp_cnt = [0]

                def emit_scores_exp(pair, h2, qch):
                    """Scores + exp for one (head, qch) unit. Yields per ktg.

                    Exp routing: ~30% of tiles go Schraudolph (DVE int
                    mul-add, Pool max/clamp) to keep ACT off the critical
                    path; the rest use ACT's native Exp."""
                    qps, kps = qk_tiles[pair]
                    base = h2 * 64
                    pbt = [prb.tile([128, 2, 512], BF16, tag="probsT",
                                    name=f"pb{kg}")
                           for kg in range(TS // 256)]
                    for ktg in range(TS // 256):
                        psc = [ps2.tile([128, 512], F32, tag="psc",
                                        name=f"psc{ktg}j{j}") for j in range(2)]
                        for j in range(2):
                            kt = 2 * ktg + j
                            nc.tensor.matmul(
                                psc[j][:],
                                kps[kt // 4][base:base + DH,
                                             (kt % 4) * 128:(kt % 4 + 1) * 128],
                                qps[qch][base:base + DH, :],
                                start=True, stop=True)
                        exp_cnt[0] += 1
                        sch_frac = 4
                        if (exp_cnt[0] * 3) % 10 < sch_frac:
                            # Schraudolph exp2: bits = y*K1+K2 (DVE),
                            # bitcast to f32, clamp negatives to 0 (Pool)
                            for j in range(2):
                                sch = schp.tile([128, 512],
                                                mybir.dt.int32, tag="sch")
                                nc.vector.tensor_scalar(
                                    out=sch[:], in0=psc[j][:],
                                    scalar1=96817625.34,
                                    scalar2=-484236300.5,
                                    op0=mybir.AluOpType.mult,
                                    op1=mybir.AluOpType.add)
                                nc.gpsimd.tensor_scalar(
                                    out=pbt[ktg][:, j, :],
                                    in0=sch[:].bitcast(F32), scalar1=0.0,
                                    scalar2=None, op0=mybir.AluOpType.max)
                        else:
                            for j in range(2):
                                nc.scalar.activation(
                                    out=pbt[ktg][:, j, :], in_=psc[j][:],
                                    func=mybir.ActivationFunctionType.Exp,
                                    scale=8.0, bias=ebias[:])
                        yield
                    yield ("unit", pair, h2, qch, pbt)

                def emit_pv_norm(pair, h2, qch, pbt, o_norm):
                    """PV + softmax-normalize for a unit whose probs are done."""
                    vg = vg_tiles[pair // 2]
                    hl = (pair * 2 + h2) % 4
                    pvt = pvp.tile([128, 4, DH + 1], F32, tag="pvt")
                    for qt in range(4):
                        for kt in range(TS // 128):
                            nc.tensor.matmul(
                                pvt[:, qt, :],
                                pbt[kt // 2][:, kt % 2,
                                             qt * 128:(qt + 1) * 128],
                                vg[:, kt, hl, :],
                                start=(kt == 0), stop=(kt == TS // 128 - 1))
                    rec = asm.tile([128, 4], F32, tag="rec")
                    nc.vector.reciprocal(out=rec[:], in_=pvt[:, :, DH])
                    nc.vector.tensor_tensor(
                        out=o_norm[:, qch * 4:qch * 4 + 4, h2, :],
                        in0=pvt[:, :, 0:DH], in1=_bcast0(rec[:], DH),
                        op=mybir.AluOpType.mult)

                def emit_oT(pair, o_norm):
                    """Transpose pair's o chunk -> oT hi/lo (c-chunk = pair)."""
                    for ch in range(2):
                        pt = pso.tile([128, 512], F32, tag="pso")
                        for i in range(4):
                            qt = 4 * ch + i
                            nc.tensor.matmul(
                                pt[:, i * 128:(i + 1) * 128],
                                o_norm[:, qt, :, :].rearrange("p h d -> p (h d)"),
                                ident_b[:], start=True, stop=True)
                        nc.vector.tensor_copy(out=oT_hi[ch][:, pair, :], in_=pt[:])
                        nc.vector.scalar_tensor_tensor(
                            out=oT_lo[ch][:, pair, :],
                            in0=pt[:], scalar=1.0,
                            in1=oT_hi[ch][:, pair, :],
                            op0=mybir.AluOpType.mult,
                            op1=mybir.AluOpType.subtract)

                def drain(gen, n=None):
                    k = 0
                    for _ in gen:
                        k += 1
                        if n is not None and k >= n:
                            return True
                    return False

                def gen_chain(g):
                    yield from qkv_gen(g)
                    yield from v_gen(g)

                cur = [None]
                nqk = [2]
                done_chain = [1]  # chains 0,1 fully emitted during stage A

                def pull_qk(pair, n):
                    for _ in range(n):
                        if cur[0] is None and nqk[0] < 4 and nqk[0] <= pair // 2 + 1:
                            cur[0] = gen_chain(nqk[0])
                            nqk[0] += 1
                        if cur[0] is None:
                            return
                        if not drain(cur[0], 1):
                            done_chain[0] = nqk[0] - 1
                            cur[0] = None

                ycnt = [0]
                o_norms = {}
                pending = [None]  # (pair, h2, qch, pbt)

                def flush_pending():
                    if pending[0] is not None:
                        p_, h2_, qch_, pbt_ = pending[0]
                        # PV contracts over every vg[p_//2] tile: the whole
                        # chain must be emitted before the PV matmuls
                        while done_chain[0] < p_ // 2:
                            pull_qk(p_, 1)
                        emit_pv_norm(p_, h2_, qch_, pbt_, o_norms[p_])
                        pending[0] = None
                        if h2_ == 1 and qch_ == TQ // 512 - 1:
                            emit_oT(p_, o_norms.pop(p_))

                for pair in range(H // 2):
                    while pair not in qk_tiles or pair // 2 not in vg_tiles:
                        pull_qk(pair, 1)
                    o_norms[pair] = onp.tile([128, TQ // 128, 2, DH], BF16,
                                             tag="o_norm", name=f"o_norm{pair}")
                    for h2 in range(2):
                        for qch in range(TQ // 512):
                            for tok in emit_scores_exp(pair, h2, qch):
                                if isinstance(tok, tuple):
                                    flush_pending()
                                    pending[0] = (pair, h2, qch, tok[4])
                                else:
                                    ycnt[0] += 1
                                    if ycnt[0] % (3 if pair < 4 else 2) == 0:
                                        pull_qk(pair, 1)
                flush_pending()

            # ============ Stage D: oT split, Wo (3-term fp8), residual, LN2 ====
            with contextlib.ExitStack() as dstk:
                x2p = dstk.enter_context(tc.tile_pool(name="x2p", bufs=1))
                xn2p = dstk.enter_context(tc.tile_pool(name="xn2p", bufs=2))
                x2 = x2p.tile([128, TQ // 128, C], BF16, tag="x2")
                xn2_hi = [xn2p.tile([128, NKO, 512], E4, tag="xn2hi",
                                    name=f"xn2hi{i}") for i in range(2)]
                xn2_lo = [xn2p.tile([128, NKO, 512], E5, tag="xn2lo",
                                    name=f"xn2lo{i}") for i in range(2)]

                b2rp = dstk.enter_context(tc.tile_pool(name="b2rp", bufs=1))
                b2r = b2rp.tile([128, C], F32, tag="b2r")
                nc.sync.dma_start(out=b2r[:], in_=b2r_d[:, :])
                pst2 = dstk.enter_context(tc.tile_pool(name="pst2", bufs=3,
                                                       space="PSUM"))
                psE = dstk.enter_context(tc.tile_pool(name="psE", bufs=3,
                                                      space="PSUM"))
                with contextlib.ExitStack() as dd:
                    wop = dd.enter_context(tc.tile_pool(name="wop", bufs=1))
                    workD = dd.enter_context(tc.tile_pool(name="workD", bufs=4))
                    wo_hi = wop.tile([128, NKO, C], E4, tag="wohi")
                    wo_lo = wop.tile([128, NKO, C], E5, tag="wolo")
                    nc.sync.dma_start(out=wo_hi[:],
                                      in_=woh_d.rearrange("(o p) f -> p o f", p=128))
                    nc.sync.dma_start(out=wo_lo[:],
                                      in_=wol_d.rearrange("(o p) f -> p o f", p=128))

                    # aoT back to token-major + residual -> x2; LN2 -> xn2 hi/lo
                    def finishD(t, mv, rstd):
                        xn2_r = workD.tile([128, C], F32R, tag="xn2_r")
                        _ln_stats_b(nc, mv, rstd, x2[:, t, :], xn2_r[:],
                                    pool=False)
                        for cg in range(2):
                            pt = pst2.tile([128, 4, 128], F32R, tag="pst2")
                            for i in range(4):
                                c = 4 * cg + i
                                nc.tensor.transpose(
                                    pt[:, i, :],
                                    xn2_r[:, c * 128:(c + 1) * 128], ident_r[:])
                            xsl = (slice(4 * cg, 4 * cg + 4),
                                   slice((t % 4) * 128, (t % 4 + 1) * 128))
                            nc.scalar.activation(
                                out=xn2_hi[t // 4][:, xsl[0], xsl[1]],
                                in_=pt[:], func=mybir.ActivationFunctionType.Copy,
                                bias=0.0, scale=1.0)
                            nc.vector.scalar_tensor_tensor(
                                out=xn2_lo[t // 4][:, xsl[0], xsl[1]],
                                in0=pt[:], scalar=1.0,
                                in1=xn2_hi[t // 4][:, xsl[0], xsl[1]],
                                op0=mybir.AluOpType.mult,
                                op1=mybir.AluOpType.subtract)

                    prevD = None
                    for t in range(TQ // 128):
                        x_t = workD.tile([128, C], BF16, tag="x_t")
                        nc.sync.dma_start(out=x_t[:],
                                          in_=xpb_d[t * 128:(t + 1) * 128, :])
                        tsl = slice((t % 4) * 128, (t % 4 + 1) * 128)
                        for half in range(2):
                            hsl = slice(half * 512, (half + 1) * 512)
                            pw = psE.tile([128, 512], F32, tag="psE")
                            for kop in range(NKO // 2):
                                ksl = slice(2 * kop, 2 * kop + 2)
                                nc.tensor.matmul(pw[:], oT_hi[t // 4][:, ksl, tsl],
                                                 wo_hi[:, ksl, hsl], perf_mode=DR,
                                                 start=(kop == 0), stop=False)
                                nc.tensor.matmul(pw[:], oT_hi[t // 4][:, ksl, tsl],
                                                 wo_lo[:, ksl, hsl], perf_mode=DR,
                                                 start=False, stop=False)
                                nc.tensor.matmul(pw[:], oT_lo[t // 4][:, ksl, tsl],
                                                 wo_hi[:, ksl, hsl], perf_mode=DR,
                                                 start=False, stop=(kop == NKO // 2 - 1))
                            nc.vector.scalar_tensor_tensor(
                                out=x2[:, t, hsl], in0=pw[:],
                                scalar=1.0 / WSCALE, in1=x_t[:, hsl],
                                op0=mybir.AluOpType.mult,
                                op1=mybir.AluOpType.add)
                        mv, rstd = _ln_stats_a(nc, stats, x2[:, t, :], eps_t)
                        if prevD is not None:
                            finishD(*prevD)
                        prevD = (t, mv, rstd)
                    finishD(*prevD)

                # ============ Stage E: FFN up (W1, relu) 3-term fp8 ============
                h1p = dstk.enter_context(tc.tile_pool(name="h1p", bufs=1))
                h1_hi = h1p.tile([128, DFF // 128, TQ], E4, tag="h1hi")
                h1_lo = h1p.tile([128, DFF // 128, TQ], E5, tag="h1lo")
                w1h_r = w1h_d.rearrange("(o p) f -> p o f", p=128)
                w1l_r = w1l_d.rearrange("(o p) f -> p o f", p=128)
                w2fp = dstk.enter_context(tc.tile_pool(name="w2fp", bufs=1))
                w2fh = w2fp.tile([128, DFF // 128, C], E4, tag="w2fh")
                w2fl = w2fp.tile([128, DFF // 128, C], E5, tag="w2fl")
                w2h_r = w2h_d.rearrange("(o p) f -> p o f", p=128)
                w2l_r = w2l_d.rearrange("(o p) f -> p o f", p=128)
                with tc.tile_pool(name="w1p", bufs=2) as w1p:
                    for blk in range(DFF // 512):
                        w1th = w1p.tile([128, NKO, 512], E4, tag="w1th")
                        w1tl = w1p.tile([128, NKO, 512], E5, tag="w1tl")
                        nc.sync.dma_start(out=w1th[:],
                                          in_=w1h_r[:, :, blk * 512:(blk + 1) * 512])
                        nc.sync.dma_start(out=w1tl[:],
                                          in_=w1l_r[:, :, blk * 512:(blk + 1) * 512])
                        # stream the full-width W2 tiles in behind the W1
                        # blocks (the DMA lane has slack during stage E)
                        if blk >= 4:
                            c4 = (blk - 4) * 256
                            nc.sync.dma_start(out=w2fh[:, :, c4:c4 + 256],
                                              in_=w2h_r[:, :, c4:c4 + 256])
                            nc.sync.dma_start(out=w2fl[:, :, c4:c4 + 256],
                                              in_=w2l_r[:, :, c4:c4 + 256])
                        for ch in range(TQ // 512):
                            csl = slice(ch * 512, (ch + 1) * 512)
                            for fs in range(4):
                                f = blk * 4 + fs
                                fsl = slice(fs * 128, (fs + 1) * 128)
                                ph = psE.tile([128, 512], F32, tag="psE")
                                for kop in range(NKO // 2):
                                    ksl = slice(2 * kop, 2 * kop + 2)
                                    nc.tensor.matmul(ph[:], w1th[:, ksl, fsl],
                                                     xn2_hi[ch][:, ksl, :], perf_mode=DR,
                                                     start=(kop == 0), stop=False)
                                    nc.tensor.matmul(ph[:], w1tl[:, ksl, fsl],
                                                     xn2_hi[ch][:, ksl, :], perf_mode=DR,
                                                     start=False, stop=False)
                                    nc.tensor.matmul(ph[:], w1th[:, ksl, fsl],
                                                     xn2_lo[ch][:, ksl, :], perf_mode=DR,
                                                     start=False,
                                                     stop=(kop == NKO // 2 - 1))
                                nc.scalar.activation(
                                    out=h1_hi[:, f, csl], in_=ph[:],
                                    func=mybir.ActivationFunctionType.Relu,
                                    bias=b1_s[:, f:f + 1], scale=1.0)
                                nc.vector.scalar_tensor_tensor(
                                    out=h1_lo[:, f, csl], in0=ph[:], scalar=0.0,
                                    in1=h1_hi[:, f, csl],
                                    op0=mybir.AluOpType.max,
                                    op1=mybir.AluOpType.subtract)

                # ============ Stage F: FFN down (W2) 3-term fp8 + residual ======
                # Token-major output: out[t, c] = h1^T @ W2 — lhsT is the
                # (already feature-major) h1, so no final transposes and the
                # residual + b2 ride the DVE evacuation directly.
                with tc.tile_pool(name="workF", bufs=3) as workF:
                    for t in range(TQ // 128):
                        out_t = workF.tile([128, C], BF16, tag="out_t")
                        tsl = slice(t * 128, (t + 1) * 128)
                        for half in range(2):
                            hsl = slice(half * 512, (half + 1) * 512)
                            po2 = psE.tile([128, 512], F32, tag="psE")
                            for kop in range(DFF // 256):
                                ksl = slice(2 * kop, 2 * kop + 2)
                                nc.tensor.matmul(po2[:], h1_hi[:, ksl, tsl],
                                                 w2fh[:, ksl, hsl], perf_mode=DR,
                                                 start=(kop == 0), stop=False)
                                nc.tensor.matmul(po2[:], h1_hi[:, ksl, tsl],
                                                 w2fl[:, ksl, hsl], perf_mode=DR,
                                                 start=False, stop=False)
                                nc.tensor.matmul(po2[:], h1_lo[:, ksl, tsl],
                                                 w2fh[:, ksl, hsl], perf_mode=DR,
                                                 start=False,
                                                 stop=(kop == DFF // 256 - 1))
                            nc.vector.scalar_tensor_tensor(
                                out=out_t[:, hsl], in0=po2[:],
                                scalar=1.0 / (WSCALE * WSCALE),
                                in1=b2r[:, hsl],
                                op0=mybir.AluOpType.mult,
                                op1=mybir.AluOpType.add)
                            nc.vector.tensor_tensor(
                                out=out_t[:, hsl], in0=out_t[:, hsl],
                                in1=x2[:, t, hsl],
                                op=mybir.AluOpType.add)
                            nc.sync.dma_start(
                                out=out_d[t * 128:(t + 1) * 128, hsl],
                                in_=out_t[:, hsl])

    nc.finalize()
    _legalize_sem_waits(nc)
    return nc


_NC_CACHE = None


def _get_nc():
    global _NC_CACHE
    if _NC_CACHE is None:
        _NC_CACHE = _build_nc()
    return _NC_CACHE


def _split_w(w, scale=WSCALE):
    ws = np.asarray(w, np.float32) * scale
    hi = ws.astype(ml_dtypes.float8_e4m3)
    lo = (ws - hi.astype(np.float32)).astype(ml_dtypes.float8_e5m2)
    return np.ascontiguousarray(hi), np.ascontiguousarray(lo)


def _shard_inputs(inputs):
    x = np.asarray(inputs["x"], np.float32)
    ln1_g = np.asarray(inputs["ln1_g"], np.float32).reshape(C)
    ln1_b = np.asarray(inputs["ln1_b"], np.float32).reshape(C)
    ln2_g = np.asarray(inputs["ln2_g"], np.float32).reshape(C)
    ln2_b = np.asarray(inputs["ln2_b"], np.float32).reshape(C)
    wq = np.ascontiguousarray(
        np.transpose(np.asarray(inputs["Wq"], np.float32), (1, 0, 2)).reshape(C, C))
    wk = np.ascontiguousarray(
        np.transpose(np.asarray(inputs["Wk"], np.float32), (1, 0, 2)).reshape(C, C))
    wv = np.ascontiguousarray(
        np.transpose(np.asarray(inputs["Wv"], np.float32), (1, 0, 2)).reshape(C, C))
    wo = np.asarray(inputs["Wo"], np.float32)
    w1 = np.asarray(inputs["W1"], np.float32)
    w2 = np.asarray(inputs["W2"], np.float32)

    # fold LN affine into the consuming weights/biases
    bq = np.asarray(inputs["bq"], np.float32).reshape(C) + ln1_b @ wq
    bk = np.asarray(inputs["bk"], np.float32).reshape(C) + ln1_b @ wk
    bv = np.asarray(inputs["bv"], np.float32).reshape(C) + ln1_b @ wv
    wq = np.ascontiguousarray(ln1_g[:, None] * wq)
    wk = np.ascontiguousarray(ln1_g[:, None] * wk)
    wv = np.ascontiguousarray(ln1_g[:, None] * wv)
    b1 = WSCALE * (np.asarray(inputs["b1"], np.float32).reshape(DFF) + ln2_b @ w1)
    assert np.abs(b1).max() == 0.0, "nonzero effective W1 bias unsupported by lo-split"
    w1g = ln2_g[:, None] * w1

    wqh, wql = _split_w(wq)
    wkh, wkl = _split_w(wk)
    wvh, wvl = _split_w(wv)
    woh, wol = _split_w(wo)
    w1h, w1l = _split_w(w1g)
    w2h, w2l = _split_w(w2)

    shared = {
        "wqh": wqh, "wql": wql, "wkh": wkh, "wkl": wkl, "wvh": wvh, "wvl": wvl,
        "woh": woh, "wol": wol, "w1h": w1h, "w1l": w1l, "w2h": w2h, "w2l": w2l,
        "bq": bq, "bk": bk,
        "bv": np.ascontiguousarray(
            np.broadcast_to(bv.astype(ml_dtypes.bfloat16), (128, C))),
        "bo": np.asarray(inputs["bo"], np.float32).reshape(C),
        "b1": b1,
        "b2": np.asarray(inputs["b2"], np.float32).reshape(C),
        "b2r": np.ascontiguousarray(np.broadcast_to(
            np.asarray(inputs["b2"], np.float32).reshape(1, C), (128, C))),
    }
    bo_v = np.asarray(inputs["bo"], np.float32).reshape(C)
    in_maps = []
    for c in range(N_CORES):
        b, half = c // 2, c % 2
        own = x[b, half * TQ:(half + 1) * TQ]
        other = x[b, (1 - half) * TQ:(2 - half) * TQ]
        x_perm = np.ascontiguousarray(np.concatenate([own, other], axis=0))
        xpb = np.ascontiguousarray((own + bo_v).astype(ml_dtypes.bfloat16))
        in_maps.append(dict(shared, x=x_perm, xpb=xpb))
    return in_maps


def _run(inputs, **spmd_kwargs):
    nc = _get_nc()
    in_maps = _shard_inputs(inputs)
    res = run_bass_kernel_spmd(nc, in_maps, core_ids=list(range(N_CORES)), **spmd_kwargs)
    out = np.empty((B, T, C), np.float32)
    for c in range(N_CORES):
        b, half = c // 2, c % 2
        out[b, half * TQ:(half + 1) * TQ] = \
            np.asarray(res.results[c]["out"]).astype(np.float32)
    return out, res


def kernel(**inputs) -> np.ndarray:
    out, _ = _run(inputs)
    return out



# revision 72
# speedup vs baseline: 1.0089x; 1.0078x over previous
"""Trainium2 Bass kernel for a pre-norm transformer block (MHSA + FFN).

Sharding: 8 cores, data parallel over (batch, seq-half). Core c handles
batch c//2, sequence half c%2. Inputs are permuted so each core's own
1024 tokens come first; attention K/V run over all 2048 tokens of the
batch (softmax is permutation invariant).

Numerics: ALL five projections (Q/K/V/Wo/W1/W2) run as 3-term
compensated fp8 DoubleRow (hi=e4m3, lo=e5m2, x@W ~= xh@Wh + xh@Wl +
xl@Wh, 4x bf16 matmul throughput per term), with weights pre-scaled x32
and split host-side; the normalized activations are split on-chip after
the TensorE transpose. Scores stay f32r (softmax logits are ~N(0,26^2)
— direct fp8 there flips argmaxes); softmax probs in bf16 with a
constant exp shift; PV runs probs-stationary so only the 65-wide
(dh+denominator) V operand streams. LayerNorm gains/biases are folded
into the downstream weights/biases host-side, so on-chip LN is pure
z-normalization.

Scheduling: the softmax exp (the largest non-matmul cost, ~218us if
ACT-only) is split ~60/40 between ACT's native Exp and a Schraudolph
exp2 (DVE int mul-add + Pool max/clamp; GPSIMD cannot read PSUM, so
the PSUM-side op must be the DVE one). Stage A (LN1+transpose+split)
streams pairs 0-3's Q/K/V matmuls in as each transposed 512-token
block completes, so the PE is busy during LN; DMA issue order is
arranged so the first x tiles are not stuck behind weight/bias loads
on the serial DMA lane.

Wo and W2 both run token-major (lhsT = the already-feature-major oT /
h1), so the attention and FFN outputs land directly in token order:
no aoT/ffnT intermediates, no transpose-back matmuls, and the
residual + bias ride the single DVE evacuation of each PSUM group
(bo is pre-added to the x residual host-side, b2 comes in replicated
across partitions). x2, the xpb residual input, and the DRAM output
are bf16 (host upconverts) to cut SBUF and serial-DMA-lane pressure.
PV safety: a chain-completion tracker drains each QKV generator fully
before the first PV that contracts over its V tiles.
"""
import contextlib

import numpy as np
import ml_dtypes

import concourse.bass as bass
import concourse.tile as tile
import concourse.mybir as mybir
from concourse.bass_utils import run_bass_kernel_spmd
from concourse.masks import make_identity

B, T, C = 4, 2048, 1024
H, DH = 16, 64
DFF = 4 * C
N_CORES = 8
TQ = T // 2          # tokens owned per core
TS = T               # key/value tokens per core
NKO = C // 128       # 8 contraction tiles for C
F32R = mybir.dt.float32r
F32 = mybir.dt.float32
BF16 = mybir.dt.bfloat16
E4 = mybir.dt.float8e4
E5 = mybir.dt.float8e5
EXP_BIAS = -128.0
EPS = 1e-5
WSCALE = 32.0
DR = mybir.MatmulPerfMode.DoubleRow

# ---------------------------------------------------------------------------
# Compat: this walrus build accepts at most 1 sem-wait per regular
# instruction (2 per InstEventSemaphore). bacc misses some tile-generated
# instructions, so split waits ourselves after finalize.
_ev_counter = [0]


def _legalize_sem_waits(nc):
    for func in nc.m.functions:
        for bb in func.blocks:
            new = []
            changed = False
            for inst in bb.instructions:
                si = inst.sync_info
                cap = 2 if isinstance(inst, mybir.InstEventSemaphore) else 1
                if si is not None and len(si.on_wait) > cap:
                    waits = list(si.on_wait)
                    for i in range(cap, len(waits), 2):
                        _ev_counter[0] += 1
                        e = mybir.InstEventSemaphore(
                            name=f"EVSPLIT-{_ev_counter[0]}", ins=[], outs=[])
                        e.engine = inst.engine
                        e.sync_info = mybir.SyncInfo(
                            on_wait=waits[i:i + 2], on_update=[])
                        new.append(e)
                    inst.sync_info = mybir.SyncInfo(
                        on_wait=waits[:cap], on_update=list(si.on_update))
                    changed = True
                new.append(inst)
            if changed:
                bb.instructions = new


# ---------------------------------------------------------------------------

def _ln_stats_a(nc, stats, x_ap, eps_t):
    """bn stats + sqrt(var+eps) for x_ap [128, C]; returns (mv, rstd)."""
    st = stats.tile([128, 2, 6], F32, tag="bnstats")
    mv = stats.tile([128, 2], F32, tag="bnaggr")
    xg = x_ap.rearrange("p (s d) -> p s d", s=2)
    for s in range(2):
        nc.vector.bn_stats(out=st[:, s, :], in_=xg[:, s, :])
    nc.vector.bn_aggr(out=mv[:], in_=st[:])
    rstd = stats.tile([128, 1], F32, tag="rstd")
    nc.scalar.activation(out=rstd[:], in_=mv[:, 1:2],
                         func=mybir.ActivationFunctionType.Sqrt,
                         bias=eps_t[:], scale=1.0)
    return mv, rstd


def _ln_stats_b(nc, mv, rstd, x_ap, out_ap, pool=False):
    """finish z-normalize: recip + (x - mu) * rstd."""
    nc.vector.reciprocal(out=rstd[:], in_=rstd[:])
    eng = nc.gpsimd if pool else nc.vector
    eng.tensor_scalar(out=out_ap, in0=x_ap,
                      scalar1=mv[:, 0:1], scalar2=rstd[:],
                      op0=mybir.AluOpType.subtract,
                      op1=mybir.AluOpType.mult)


def _bcast0(ap, free):
    """Broadcast a [128, n] AP along a new stride-0 free dim of size `free`."""
    return bass.AP(tensor=ap.tensor, offset=ap.offset,
                   ap=[list(d) for d in ap.ap] + [[0, free]])


def _build_nc():
    nc = bass.Bass()

    # ---- I/O ----
    x_d = nc.dram_tensor("x", [T, C], F32, kind="ExternalInput")
    xpb_d = nc.dram_tensor("xpb", [TQ, C], BF16, kind="ExternalInput")
    wqh_d = nc.dram_tensor("wqh", [C, C], E4, kind="ExternalInput")
    wql_d = nc.dram_tensor("wql", [C, C], E5, kind="ExternalInput")
    wkh_d = nc.dram_tensor("wkh", [C, C], E4, kind="ExternalInput")
    wkl_d = nc.dram_tensor("wkl", [C, C], E5, kind="ExternalInput")
    wvh_d = nc.dram_tensor("wvh", [C, C], E4, kind="ExternalInput")
    wvl_d = nc.dram_tensor("wvl", [C, C], E5, kind="ExternalInput")
    woh_d = nc.dram_tensor("woh", [C, C], E4, kind="ExternalInput")
    wol_d = nc.dram_tensor("wol", [C, C], E5, kind="ExternalInput")
    w1h_d = nc.dram_tensor("w1h", [C, DFF], E4, kind="ExternalInput")
    w1l_d = nc.dram_tensor("w1l", [C, DFF], E5, kind="ExternalInput")
    w2h_d = nc.dram_tensor("w2h", [DFF, C], E4, kind="ExternalInput")
    w2l_d = nc.dram_tensor("w2l", [DFF, C], E5, kind="ExternalInput")
    bq_d = nc.dram_tensor("bq", [C], F32, kind="ExternalInput")
    bk_d = nc.dram_tensor("bk", [C], F32, kind="ExternalInput")
    bv_d = nc.dram_tensor("bv", [128, C], BF16, kind="ExternalInput")
    bo_d = nc.dram_tensor("bo", [C], F32, kind="ExternalInput")
    b1_d = nc.dram_tensor("b1", [DFF], F32, kind="ExternalInput")
    b2_d = nc.dram_tensor("b2", [C], F32, kind="ExternalInput")
    b2r_d = nc.dram_tensor("b2r", [128, C], F32, kind="ExternalInput")
    out_d = nc.dram_tensor("out", [TQ, C], BF16, kind="ExternalOutput")

    wqh_r = wqh_d.rearrange("(o p) f -> p o f", p=128)
    wql_r = wql_d.rearrange("(o p) f -> p o f", p=128)
    wkh_r = wkh_d.rearrange("(o p) f -> p o f", p=128)
    wkl_r = wkl_d.rearrange("(o p) f -> p o f", p=128)
    wvh_r = wvh_d.rearrange("(o p) f -> p o f", p=128)
    wvl_r = wvl_d.rearrange("(o p) f -> p o f", p=128)

    with tile.TileContext(nc) as tc:
        with contextlib.ExitStack() as top:
            consts = top.enter_context(tc.tile_pool(name="consts", bufs=1))
            ps = top.enter_context(tc.tile_pool(name="ps", bufs=2, space="PSUM"))
            stats = top.enter_context(tc.tile_pool(name="stats", bufs=8))

            ident_b = consts.tile([128, 128], BF16, tag="identb")
            make_identity(nc, ident_b)
            ident_r = consts.tile([128, 128], F32R, tag="identr")
            nc.vector.tensor_copy(out=ident_r[:], in_=ident_b[:])
            ebias = consts.tile([128, 1], F32, tag="ebias")
            nc.vector.memset(ebias[:], EXP_BIAS)
            eps_t = consts.tile([128, 1], F32, tag="eps")
            nc.vector.memset(eps_t[:], EPS)
            bq_s = consts.tile([128, NKO], F32, tag="bq")
            bk_s = consts.tile([128, NKO], F32, tag="bk")
            bo_s = consts.tile([128, NKO], F32, tag="bo")
            b2_s = consts.tile([128, NKO], F32, tag="b2")
            b1_s = consts.tile([128, DFF // 128], F32, tag="b1")
            bv_r = consts.tile([128, C], BF16, tag="bvr")

            def load_biases():
                for dst, src_ in ((bq_s, bq_d), (bk_s, bk_d), (bo_s, bo_d),
                                  (b2_s, b2_d), (b1_s, b1_d)):
                    nc.sync.dma_start(out=dst[:],
                                      in_=src_.rearrange("(o p) -> p o", p=128))
                nc.sync.dma_start(out=bv_r[:], in_=bv_d[:, :])

            # ============ Stages A-C: LN1, QKV, attention ============
            with contextlib.ExitStack() as abc:
                xnp = abc.enter_context(tc.tile_pool(name="xnp", bufs=8))
                xnT_hi = [xnp.tile([128, NKO, 512], E4, tag="xnThi",
                                   name=f"xnThi{i}") for i in range(4)]
                xnT_lo = [xnp.tile([128, NKO, 512], E5, tag="xnTlo",
                                   name=f"xnTlo{i}") for i in range(4)]
                wgp = abc.enter_context(tc.tile_pool(name="wgp", bufs=2))
                qkp = abc.enter_context(tc.tile_pool(name="qkp", bufs=2))
                vgp = abc.enter_context(tc.tile_pool(name="vgp", bufs=2))

                qk_tiles = {}
                vg_tiles = {}
                wq_tiles = {}
                wv_tiles = {}

                def load_qk_w(g):
                    wqt_h = wgp.tile([128, NKO, 256], E4, tag="wqth")
                    wqt_l = wgp.tile([128, NKO, 256], E5, tag="wqtl")
                    wkt_h = wgp.tile([128, NKO, 256], E4, tag="wkth")
                    wkt_l = wgp.tile([128, NKO, 256], E5, tag="wktl")
                    fsl_w = slice(g * 256, (g + 1) * 256)
                    nc.sync.dma_start(out=wqt_h[:], in_=wqh_r[:, :, fsl_w])
                    nc.sync.dma_start(out=wqt_l[:], in_=wql_r[:, :, fsl_w])
                    nc.sync.dma_start(out=wkt_h[:], in_=wkh_r[:, :, fsl_w])
                    nc.sync.dma_start(out=wkt_l[:], in_=wkl_r[:, :, fsl_w])
                    wq_tiles[g] = (wqt_h, wqt_l, wkt_h, wkt_l)

                def load_v_w(g):
                    wvt_h = wgp.tile([128, NKO, 256], E4, tag="wvth")
                    wvt_l = wgp.tile([128, NKO, 256], E5, tag="wvtl")
                    nc.sync.dma_start(out=wvt_h[:], in_=wvh_r[:, :, g * 256:(g + 1) * 256])
                    nc.sync.dma_start(out=wvt_l[:], in_=wvl_r[:, :, g * 256:(g + 1) * 256])
                    wv_tiles[g] = (wvt_h, wvt_l)
                    vg = vgp.tile([128, TS // 128, 4, 65], BF16, tag="vg")
                    vg_tiles[g] = vg
                    nc.vector.memset(vg[:, :, :, DH:DH + 1], 1.0)

                def alloc_qk(pair):
                    i = pair % 2
                    qps = [qkp.tile([128, 512], F32R, tag=f"qp{i}c{ch}",
                                    name=f"qp{pair}c{ch}")
                           for ch in range(TQ // 512)]
                    kps = [qkp.tile([128, 512], F32R, tag=f"kp{i}c{ch}",
                                    name=f"kp{pair}c{ch}")
                           for ch in range(TS // 512)]
                    qk_tiles[pair] = (qps, kps)

                def qk_group(pair, kind, ch, on_act=False):
                    """One Q or K psum group (3-term fp8 DR) + evacuation."""
                    g, i = pair // 2, pair % 2
                    wqt_h, wqt_l, wkt_h, wkt_l = wq_tiles[g]
                    wh, wl = (wqt_h, wqt_l) if kind == 'q' else (wkt_h, wkt_l)
                    dst = qk_tiles[pair][0 if kind == 'q' else 1][ch]
                    bias = bq_s if kind == 'q' else bk_s
                    isl = slice(i * 128, (i + 1) * 128)
                    pq = ps.tile([128, 512], F32, tag="ps")
                    for kop in range(NKO // 2):
                        ksl = slice(2 * kop, 2 * kop + 2)
                        nc.tensor.matmul(pq[:], wh[:, ksl, isl],
                                         xnT_hi[ch][:, ksl, :], perf_mode=DR,
                                         start=(kop == 0), stop=False)
                        nc.tensor.matmul(pq[:], wl[:, ksl, isl],
                                         xnT_hi[ch][:, ksl, :], perf_mode=DR,
                                         start=False, stop=False)
                        nc.tensor.matmul(pq[:], wh[:, ksl, isl],
                                         xnT_lo[ch][:, ksl, :], perf_mode=DR,
                                         start=False, stop=(kop == NKO // 2 - 1))
                    if on_act:
                        nc.scalar.activation(
                            out=dst[:], in_=pq[:],
                            func=mybir.ActivationFunctionType.Identity,
                            bias=bias[:, pair:pair + 1], scale=1.0 / WSCALE)
                    else:
                        nc.vector.tensor_scalar(
                            out=dst[:], in0=pq[:],
                            scalar1=1.0 / WSCALE, scalar2=bias[:, pair:pair + 1],
                            op0=mybir.AluOpType.mult,
                            op1=mybir.AluOpType.add)

                def v_tile(g, to):
                    wvt_h, wvt_l = wv_tiles[g]
                    vg = vg_tiles[g]
                    pv = ps.tile([128, 512], F32, tag="ps")
                    tsl = slice((to % 4) * 128, (to % 4 + 1) * 128)
                    # V runs 2-term (xh@Wh + xh@Wl = xh@W): V is bf16-rounded
                    # right after anyway, and errors average through the
                    # softmax mixture, so the xl term isn't worth its cycles.
                    for kop in range(NKO // 2):
                        ksl = slice(2 * kop, 2 * kop + 2)
                        nc.tensor.matmul(pv[0:128, 0:256],
                                         xnT_hi[to // 4][:, ksl, tsl],
                                         wvt_h[:, ksl, :], perf_mode=DR,
                                         start=(kop == 0), stop=False)
                        nc.tensor.matmul(pv[0:128, 0:256],
                                         xnT_hi[to // 4][:, ksl, tsl],
                                         wvt_l[:, ksl, :], perf_mode=DR,
                                         start=False, stop=(kop == NKO // 2 - 1))
                    nc.vector.scalar_tensor_tensor(
                        out=vg[:, to, :, 0:DH],
                        in0=pv[:, 0:256].rearrange("p (h d) -> p h d", d=DH),
                        scalar=1.0 / WSCALE,
                        in1=bv_r[:, g * 256:(g + 1) * 256].rearrange(
                            "p (h d) -> p h d", d=DH),
                        op0=mybir.AluOpType.mult,
                        op1=mybir.AluOpType.add)

                def head_block(b):
                    """Emit all pair-0..3 QKV units that only need xnT block b.
                    Q/K evacuations ride ACT here (DVE is stage-A-loaded)."""
                    if b < 2:
                        for pair in range(4):
                            qk_group(pair, 'q', b, on_act=True)
                            qk_group(pair, 'k', b, on_act=True)
                    else:
                        for pair in range(4):
                            qk_group(pair, 'k', b, on_act=True)
                    for g in range(2):
                        for to in range(4 * b, 4 * b + 4):
                            v_tile(g, to)

                # ---- Stage A: LN1 (z-norm only) + transpose -> xnT hi/lo,
                # with pairs 0-3 QKV streaming in as blocks complete ----
                with tc.tile_pool(name="workA", bufs=4) as workA, \
                     tc.tile_pool(name="pstA", bufs=2, space="PSUM") as pstA:
                    def finishA(t, x_t, mv, rstd):
                        xn_r = workA.tile([128, C], F32R, tag="xn_r")
                        _ln_stats_b(nc, mv, rstd, x_t[:], xn_r[:], pool=True)
                        for cg in range(2):
                            pt = pstA.tile([128, 4, 128], F32R, tag="pstA")
                            for i in range(4):
                                nc.tensor.transpose(
                                    pt[:, i, :],
                                    xn_r[:, (4 * cg + i) * 128:(4 * cg + i + 1) * 128],
                                    ident_r[:])
                            xsl = (slice(4 * cg, 4 * cg + 4),
                                   slice((t % 4) * 128, (t % 4 + 1) * 128))
                            nc.scalar.activation(
                                out=xnT_hi[t // 4][:, xsl[0], xsl[1]],
                                in_=pt[:],
                                func=mybir.ActivationFunctionType.Copy,
                                bias=0.0, scale=1.0)
                            nc.vector.scalar_tensor_tensor(
                                out=xnT_lo[t // 4][:, xsl[0], xsl[1]],
                                in0=pt[:], scalar=1.0,
                                in1=xnT_hi[t // 4][:, xsl[0], xsl[1]],
                                op0=mybir.AluOpType.mult,
                                op1=mybir.AluOpType.subtract)

                    for pair in range(4):
                        alloc_qk(pair)
                    prevA = None
                    for t in range(T // 128):
                        x_t = workA.tile([128, C], F32, tag="x_t")
                        nc.sync.dma_start(out=x_t[:], in_=x_d[t * 128:(t + 1) * 128, :])
                        if t == 0:
                            load_biases()
                        elif t == 1:
                            load_qk_w(0)
                        elif t == 2:
                            load_v_w(0)
                        elif t == 3:
                            load_qk_w(1)
                            load_v_w(1)
                        mv, rstd = _ln_stats_a(nc, stats, x_t[:], eps_t)
                        if prevA is not None:
                            finishA(*prevA)
                            if prevA[0] % 4 == 3:
                                head_block(prevA[0] // 4)
                        prevA = (t, x_t, mv, rstd)
                    finishA(*prevA)
                    head_block(3)

                # ---- Stages B+C interleaved ----
                prb = abc.enter_context(tc.tile_pool(name="probs", bufs=12))
                onp = abc.enter_context(tc.tile_pool(name="onp", bufs=2))
                otp = abc.enter_context(tc.tile_pool(name="otp", bufs=2))
                pvp = abc.enter_context(tc.tile_pool(name="pvp", bufs=1, space="PSUM"))
                pso = abc.enter_context(tc.tile_pool(name="pso", bufs=1, space="PSUM"))
                ps2 = abc.enter_context(tc.tile_pool(name="ps2", bufs=4, space="PSUM"))
                asm = abc.enter_context(tc.tile_pool(name="att_sm", bufs=3))
                schp = abc.enter_context(tc.tile_pool(name="schp", bufs=3))

                oT_hi = [otp.tile([128, NKO, 512], E4, tag="oThi",
                                  name=f"oThi{i}") for i in range(2)]
                oT_lo = [otp.tile([128, NKO, 512], E5, tag="oTlo",
                                  name=f"oTlo{i}") for i in range(2)]

                def qkv_gen(g):
                    """Q/K for pairs 2g, 2g+1. Yields after each psum group."""
                    load_qk_w(g)
                    for i in range(2):
                        pair = 2 * g + i
                        alloc_qk(pair)
                        for ch in range(TQ // 512):
                            qk_group(pair, 'q', ch)
                            yield
                        for ch in range(TS // 512):
                            qk_group(pair, 'k', ch)
                            yield

                def v_gen(g):
                    """V for heads 4g..4g+3 -> vg tile [128, 16, 4, 65] bf16."""
                    load_v_w(g)
                    for to in range(TS // 128):
                        v_tile(g, to)
                        yield

                exp_cnt = [0]

                def emit_scores_exp(pair, h2, qch):
                    """Scores + exp for one (head, qch) unit. Yields per ktg.

                    Exp routing: ~30% of tiles go Schraudolph (DVE int
                    mul-add, Pool max/clamp) to keep ACT off the critical
                    path; the rest use ACT's native Exp."""
                    qps, kps = qk_tiles[pair]
                    base = h2 * 64
                    pbt = [prb.tile([128, 2, 512], BF16, tag="probsT",
                                    name=f"pb{kg}")
                           for kg in range(TS // 256)]
                    for ktg in range(TS // 256):
                        psc = [ps2.tile([128, 512], F32, tag="psc",
                                        name=f"psc{ktg}j{j}") for j in range(2)]
                        for j in range(2):
                            kt = 2 * ktg + j
                            nc.tensor.matmul(
                                psc[j][:],
                                kps[kt // 4][base:base + DH,
                                             (kt % 4) * 128:(kt % 4 + 1) * 128],
                                qps[qch][base:base + DH, :],
                                start=True, stop=True)
                        exp_cnt[0] += 1
                        sch_frac = 4
                        if (exp_cnt[0] * 3) % 10 < sch_frac:
                            # Schraudolph exp2: bits = y*K1+K2 (DVE),
                            # bitcast to f32, clamp negatives to 0 (Pool)
                            for j in range(2):
                                sch = schp.tile([128, 512],
                                                mybir.dt.int32, tag="sch")
                                nc.vector.tensor_scalar(
                                    out=sch[:], in0=psc[j][:],
                                    scalar1=96817625.34,
                                    scalar2=-484236300.5,
                                    op0=mybir.AluOpType.mult,
                                    op1=mybir.AluOpType.add)
                                nc.gpsimd.tensor_scalar(
                                    out=pbt[ktg][:, j, :],
                                    in0=sch[:].bitcast(F32), scalar1=0.0,
                                    scalar2=None, op0=mybir.AluOpType.max)
                        else:
                            for j in range(2):
                                nc.scalar.activation(
                                    out=pbt[ktg][:, j, :], in_=psc[j][:],
                                    func=mybir.ActivationFunctionType.Exp,
                                    scale=8.0, bias=ebias[:])
                        yield
                    yield ("unit", pair, h2, qch, pbt)

                def emit_pv_norm(pair, h2, qch, pbt, o_norm):
                    """PV + softmax-normalize for a unit whose probs are done."""
                    vg = vg_tiles[pair // 2]
                    hl = (pair * 2 + h2) % 4
                    pvt = pvp.tile([128, 4, DH + 1], F32, tag="pvt")
                    for qt in range(4):
                        for kt in range(TS // 128):
                            nc.tensor.matmul(
                                pvt[:, qt, :],
                                pbt[kt // 2][:, kt % 2,
                                             qt * 128:(qt + 1) * 128],
                                vg[:, kt, hl, :],
                                start=(kt == 0), stop=(kt == TS // 128 - 1))
                    rec = asm.tile([128, 4], F32, tag="rec")
                    nc.vector.reciprocal(out=rec[:], in_=pvt[:, :, DH])
                    nc.vector.tensor_tensor(
                        out=o_norm[:, qch * 4:qch * 4 + 4, h2, :],
                        in0=pvt[:, :, 0:DH], in1=_bcast0(rec[:], DH),
                        op=mybir.AluOpType.mult)

                def emit_oT(pair, o_norm):
                    """Transpose pair's o chunk -> oT hi/lo (c-chunk = pair)."""
                    for ch in range(2):
                        pt = pso.tile([128, 512], F32, tag="pso")
                        for i in range(4):
                            qt = 4 * ch + i
                            nc.tensor.matmul(
                                pt[:, i * 128:(i + 1) * 128],
                                o_norm[:, qt, :, :].rearrange("p h d -> p (h d)"),
                                ident_b[:], start=True, stop=True)
                        nc.vector.tensor_copy(out=oT_hi[ch][:, pair, :], in_=pt[:])
                        nc.vector.scalar_tensor_tensor(
                            out=oT_lo[ch][:, pair, :],
                            in0=pt[:], scalar=1.0,
                            in1=oT_hi[ch][:, pair, :],
                            op0=mybir.AluOpType.mult,
                            op1=mybir.AluOpType.subtract)

                def drain(gen, n=None):
                    k = 0
                    for _ in gen:
                        k += 1
                        if n is not None and k >= n:
                            return True
                    return False

                def gen_chain(g):
                    yield from qkv_gen(g)
                    yield from v_gen(g)

                cur = [None]
                nqk = [2]
                done_chain = [1]  # chains 0,1 fully emitted during stage A

                def pull_qk(pair, n):
                    for _ in range(n):
                        if cur[0] is None and nqk[0] < 4 and nqk[0] <= pair // 2 + 1:
                            cur[0] = gen_chain(nqk[0])
                            nqk[0] += 1
                        if cur[0] is None:
                            return
                        if not drain(cur[0], 1):
                            done_chain[0] = nqk[0] - 1
                            cur[0] = None

                ycnt = [0]
                o_norms = {}
                pending = [None]  # (pair, h2, qch, pbt)

                def flush_pending():
                    if pending[0] is not None:
                        p_, h2_, qch_, pbt_ = pending[0]
                        # PV contracts over every vg[p_//2] tile: the whole
                        # chain must be emitted before the PV matmuls
                        while done_chain[0] < p_ // 2:
                            pull_qk(p_, 1)
                        emit_pv_norm(p_, h2_, qch_, pbt_, o_norms[p_])
                        pending[0] = None
                        if h2_ == 1 and qch_ == TQ // 512 - 1:
                            emit_oT(p_, o_norms.pop(p_))

                for pair in range(H // 2):
                    while pair not in qk_tiles or pair // 2 not in vg_tiles:
                        pull_qk(pair, 1)
                    o_norms[pair] = onp.tile([128, TQ // 128, 2, DH], BF16,
                                             tag="o_norm", name=f"o_norm{pair}")
                    for h2 in range(2):
                        for qch in range(TQ // 512):
                            for tok in emit_scores_exp(pair, h2, qch):
                                if isinstance(tok, tuple):
                                    flush_pending()
                                    pending[0] = (pair, h2, qch, tok[4])
                                else:
                                    ycnt[0] += 1
                                    if ycnt[0] % (3 if pair < 4 else 2) == 0:
                                        pull_qk(pair, 1)
                flush_pending()

            # ============ Stage D: oT split, Wo (3-term fp8), residual, LN2 ====
            with contextlib.ExitStack() as dstk:
                x2p = dstk.enter_context(tc.tile_pool(name="x2p", bufs=1))
                xn2p = dstk.enter_context(tc.tile_pool(name="xn2p", bufs=2))
                x2 = x2p.tile([128, TQ // 128, C], BF16, tag="x2")
                xn2_hi = [xn2p.tile([128, NKO, 512], E4, tag="xn2hi",
                                    name=f"xn2hi{i}") for i in range(2)]
                xn2_lo = [xn2p.tile([128, NKO, 512], E5, tag="xn2lo",
                                    name=f"xn2lo{i}") for i in range(2)]

                b2rp = dstk.enter_context(tc.tile_pool(name="b2rp", bufs=1))
                b2r = b2rp.tile([128, C], F32, tag="b2r")
                nc.sync.dma_start(out=b2r[:], in_=b2r_d[:, :])
                pst2 = dstk.enter_context(tc.tile_pool(name="pst2", bufs=3,
                                                       space="PSUM"))
                psE = dstk.enter_context(tc.tile_pool(name="psE", bufs=3,
                                                      space="PSUM"))
                with contextlib.ExitStack() as dd:
                    wop = dd.enter_context(tc.tile_pool(name="wop", bufs=1))
                    workD = dd.enter_context(tc.tile_pool(name="workD", bufs=4))
                    wo_hi = wop.tile([128, NKO, C], E4, tag="wohi")
                    wo_lo = wop.tile([128, NKO, C], E5, tag="wolo")
                    nc.sync.dma_start(out=wo_hi[:],
                                      in_=woh_d.rearrange("(o p) f -> p o f", p=128))
                    nc.sync.dma_start(out=wo_lo[:],
                                      in_=wol_d.rearrange("(o p) f -> p o f", p=128))

                    # aoT back to token-major + residual -> x2; LN2 -> xn2 hi/lo
                    def finishD(t, mv, rstd):
                        xn2_r = workD.tile([128, C], F32R, tag="xn2_r")
                        _ln_stats_b(nc, mv, rstd, x2[:, t, :], xn2_r[:],
                                    pool=False)
                        for cg in range(2):
                            pt = pst2.tile([128, 4, 128], F32R, tag="pst2")
                            for i in range(4):
                                c = 4 * cg + i
                                nc.tensor.transpose(
                                    pt[:, i, :],
                                    xn2_r[:, c * 128:(c + 1) * 128], ident_r[:])
                            xsl = (slice(4 * cg, 4 * cg + 4),
                                   slice((t % 4) * 128, (t % 4 + 1) * 128))
                            nc.scalar.activation(
                                out=xn2_hi[t // 4][:, xsl[0], xsl[1]],
                                in_=pt[:], func=mybir.ActivationFunctionType.Copy,
                                bias=0.0, scale=1.0)
                            nc.vector.scalar_tensor_tensor(
                                out=xn2_lo[t // 4][:, xsl[0], xsl[1]],
                                in0=pt[:], scalar=1.0,
                                in1=xn2_hi[t // 4][:, xsl[0], xsl[1]],
                                op0=mybir.AluOpType.mult,
                                op1=mybir.AluOpType.subtract)

                    prevD = None
                    for t in range(TQ // 128):
                        x_t = workD.tile([128, C], BF16, tag="x_t")
                        nc.sync.dma_start(out=x_t[:],
                                          in_=xpb_d[t * 128:(t + 1) * 128, :])
                        tsl = slice((t % 4) * 128, (t % 4 + 1) * 128)
                        for half in range(2):
                            hsl = slice(half * 512, (half + 1) * 512)
                            pw = psE.tile([128, 512], F32, tag="psE")
                            for kop in range(NKO // 2):
                                ksl = slice(2 * kop, 2 * kop + 2)
                                nc.tensor.matmul(pw[:], oT_hi[t // 4][:, ksl, tsl],
                                                 wo_hi[:, ksl, hsl], perf_mode=DR,
                                                 start=(kop == 0), stop=False)
                                nc.tensor.matmul(pw[:], oT_hi[t // 4][:, ksl, tsl],
                                                 wo_lo[:, ksl, hsl], perf_mode=DR,
                                                 start=False, stop=False)
                                nc.tensor.matmul(pw[:], oT_lo[t // 4][:, ksl, tsl],
                                                 wo_hi[:, ksl, hsl], perf_mode=DR,
                                                 start=False, stop=(kop == NKO // 2 - 1))
                            nc.vector.scalar_tensor_tensor(
                                out=x2[:, t, hsl], in0=pw[:],
                                scalar=1.0 / WSCALE, in1=x_t[:, hsl],
                                op0=mybir.AluOpType.mult,
                                op1=mybir.AluOpType.add)
                        mv, rstd = _ln_stats_a(nc, stats, x2[:, t, :], eps_t)
                        if prevD is not None:
                            finishD(*prevD)
                        prevD = (t, mv, rstd)
                    finishD(*prevD)

                # ============ Stage E: FFN up (W1, relu) 3-term fp8 ============
                h1p = dstk.enter_context(tc.tile_pool(name="h1p", bufs=1))
                h1_hi = h1p.tile([128, DFF // 128, TQ], E4, tag="h1hi")
                h1_lo = h1p.tile([128, DFF // 128, TQ], E5, tag="h1lo")
                w1h_r = w1h_d.rearrange("(o p) f -> p o f", p=128)
                w1l_r = w1l_d.rearrange("(o p) f -> p o f", p=128)
                w2fp = dstk.enter_context(tc.tile_pool(name="w2fp", bufs=1))
                w2fh = w2fp.tile([128, DFF // 128, C], E4, tag="w2fh")
                w2fl = w2fp.tile([128, DFF // 128, C], E5, tag="w2fl")
                w2h_r = w2h_d.rearrange("(o p) f -> p o f", p=128)
                w2l_r = w2l_d.rearrange("(o p) f -> p o f", p=128)
                with tc.tile_pool(name="w1p", bufs=2) as w1p:
                    for blk in range(DFF // 512):
                        w1th = w1p.tile([128, NKO, 512], E4, tag="w1th")
                        w1tl = w1p.tile([128, NKO, 512], E5, tag="w1tl")
                        nc.sync.dma_start(out=w1th[:],
                                          in_=w1h_r[:, :, blk * 512:(blk + 1) * 512])
                        nc.sync.dma_start(out=w1tl[:],
                                          in_=w1l_r[:, :, blk * 512:(blk + 1) * 512])
                        # stream the full-width W2 tiles in behind the W1
                        # blocks (the DMA lane has slack during stage E)
                        if blk >= 4:
                            c4 = (blk - 4) * 256
                            nc.sync.dma_start(out=w2fh[:, :, c4:c4 + 256],
                                              in_=w2h_r[:, :, c4:c4 + 256])
                            nc.sync.dma_start(out=w2fl[:, :, c4:c4 + 256],
                                              in_=w2l_r[:, :, c4:c4 + 256])
                        for ch in range(TQ // 512):
                            csl = slice(ch * 512, (ch + 1) * 512)
                            for fs in range(4):
                                f = blk * 4 + fs
                                fsl = slice(fs * 128, (fs + 1) * 128)
                                ph = psE.tile([128, 512], F32, tag="psE")
                                for kop in range(NKO // 2):
                                    ksl = slice(2 * kop, 2 * kop + 2)
                                    nc.tensor.matmul(ph[:], w1th[:, ksl, fsl],
                                                     xn2_hi[ch][:, ksl, :], perf_mode=DR,
                                                     start=(kop == 0), stop=False)
                                    nc.tensor.matmul(ph[:], w1tl[:, ksl, fsl],
                                                     xn2_hi[ch][:, ksl, :], perf_mode=DR,
                                                     start=False, stop=False)
                                    nc.tensor.matmul(ph[:], w1th[:, ksl, fsl],
                                                     xn2_lo[ch][:, ksl, :], perf_mode=DR,
                                                     start=False,
                                                     stop=(kop == NKO // 2 - 1))
                                nc.scalar.activation(
                                    out=h1_hi[:, f, csl], in_=ph[:],
                                    func=mybir.ActivationFunctionType.Relu,
                                    bias=b1_s[:, f:f + 1], scale=1.0)
                                nc.vector.scalar_tensor_tensor(
                                    out=h1_lo[:, f, csl], in0=ph[:], scalar=0.0,
                                    in1=h1_hi[:, f, csl],
                                    op0=mybir.AluOpType.max,
                                    op1=mybir.AluOpType.subtract)

                # ============ Stage F: FFN down (W2) 3-term fp8 + residual ======
                # Token-major output: out[t, c] = h1^T @ W2 — lhsT is the
                # (already feature-major) h1, so no final transposes and the
                # residual + b2 ride the DVE evacuation directly.
                with tc.tile_pool(name="workF", bufs=3) as workF:
                    for t in range(TQ // 128):
                        out_t = workF.tile([128, C], BF16, tag="out_t")
                        tsl = slice(t * 128, (t + 1) * 128)
                        for half in range(2):
                            hsl = slice(half * 512, (half + 1) * 512)
                            po2 = psE.tile([128, 512], F32, tag="psE")
                            for kop in range(DFF // 256):
                                ksl = slice(2 * kop, 2 * kop + 2)
                                nc.tensor.matmul(po2[:], h1_hi[:, ksl, tsl],
                                                 w2fh[:, ksl, hsl], perf_mode=DR,
                                                 start=(kop == 0), stop=False)
                                nc.tensor.matmul(po2[:], h1_hi[:, ksl, tsl],
                                                 w2fl[:, ksl, hsl], perf_mode=DR,
                                                 start=False, stop=False)
                                nc.tensor.matmul(po2[:], h1_lo[:, ksl, tsl],
                                                 w2fh[:, ksl, hsl], perf_mode=DR,
                                                 start=False,
                                                 stop=(kop == DFF // 256 - 1))
                            nc.vector.scalar_tensor_tensor(
                                out=out_t[:, hsl], in0=po2[:],
                                scalar=1.0 / (WSCALE * WSCALE),
                                in1=b2r[:, hsl],
                                op0=mybir.AluOpType.mult,
                                op1=mybir.AluOpType.add)
                            nc.vector.tensor_tensor(
                                out=out_t[:, hsl], in0=out_t[:, hsl],
                                in1=x2[:, t, hsl],
                                op=mybir.AluOpType.add)
                            nc.sync.dma_start(
                                out=out_d[t * 128:(t + 1) * 128, hsl],
                                in_=out_t[:, hsl])

    nc.finalize()
    _legalize_sem_waits(nc)
    return nc


_NC_CACHE = None


def _get_nc():
    global _NC_CACHE
    if _NC_CACHE is None:
        _NC_CACHE = _build_nc()
    return _NC_CACHE


def _split_w(w, scale=WSCALE):
    ws = np.asarray(w, np.float32) * scale
    hi = ws.astype(ml_dtypes.float8_e4m3)
    lo = (ws - hi.astype(np.float32)).astype(ml_dtypes.float8_e5m2)
    return np.ascontiguousarray(hi), np.ascontiguousarray(lo)


def _shard_inputs(inputs):
    x = np.asarray(inputs["x"], np.float32)
    ln1_g = np.asarray(inputs["ln1_g"], np.float32).reshape(C)
    ln1_b = np.asarray(inputs["ln1_b"], np.float32).reshape(C)
    ln2_g = np.asarray(inputs["ln2_g"], np.float32).reshape(C)
    ln2_b = np.asarray(inputs["ln2_b"], np.float32).reshape(C)
    wq = np.ascontiguousarray(
        np.transpose(np.asarray(inputs["Wq"], np.float32), (1, 0, 2)).reshape(C, C))
    wk = np.ascontiguousarray(
        np.transpose(np.asarray(inputs["Wk"], np.float32), (1, 0, 2)).reshape(C, C))
    wv = np.ascontiguousarray(
        np.transpose(np.asarray(inputs["Wv"], np.float32), (1, 0, 2)).reshape(C, C))
    wo = np.asarray(inputs["Wo"], np.float32)
    w1 = np.asarray(inputs["W1"], np.float32)
    w2 = np.asarray(inputs["W2"], np.float32)

    # fold LN affine into the consuming weights/biases
    bq = np.asarray(inputs["bq"], np.float32).reshape(C) + ln1_b @ wq
    bk = np.asarray(inputs["bk"], np.float32).reshape(C) + ln1_b @ wk
    bv = np.asarray(inputs["bv"], np.float32).reshape(C) + ln1_b @ wv
    wq = np.ascontiguousarray(ln1_g[:, None] * wq)
    wk = np.ascontiguousarray(ln1_g[:, None] * wk)
    wv = np.ascontiguousarray(ln1_g[:, None] * wv)
    b1 = WSCALE * (np.asarray(inputs["b1"], np.float32).reshape(DFF) + ln2_b @ w1)
    assert np.abs(b1).max() == 0.0, "nonzero effective W1 bias unsupported by lo-split"
    w1g = ln2_g[:, None] * w1

    wqh, wql = _split_w(wq)
    wkh, wkl = _split_w(wk)
    wvh, wvl = _split_w(wv)
    woh, wol = _split_w(wo)
    w1h, w1l = _split_w(w1g)
    w2h, w2l = _split_w(w2)

    shared = {
        "wqh": wqh, "wql": wql, "wkh": wkh, "wkl": wkl, "wvh": wvh, "wvl": wvl,
        "woh": woh, "wol": wol, "w1h": w1h, "w1l": w1l, "w2h": w2h, "w2l": w2l,
        "bq": bq, "bk": bk,
        "bv": np.ascontiguousarray(
            np.broadcast_to(bv.astype(ml_dtypes.bfloat16), (128, C))),
        "bo": np.asarray(inputs["bo"], np.float32).reshape(C),
        "b1": b1,
        "b2": np.asarray(inputs["b2"], np.float32).reshape(C),
        "b2r": np.ascontiguousarray(np.broadcast_to(
            np.asarray(inputs["b2"], np.float32).reshape(1, C), (128, C))),
    }
    bo_v = np.asarray(inputs["bo"], np.float32).reshape(C)
    in_maps = []
    for c in range(N_CORES):
        b, half = c // 2, c % 2
        own = x[b, half * TQ:(half + 1) * TQ]
        other = x[b, (1 - half) * TQ:(2 - half) * TQ]
        x_perm = np.ascontiguousarray(np.concatenate([own, other], axis=0))
        xpb = np.ascontiguousarray((own + bo_v).astype(ml_dtypes.bfloat16))
        in_maps.append(dict(shared, x=x_perm, xpb=xpb))
    return in_maps


def _run(inputs, **spmd_kwargs):
    nc = _get_nc()
    in_maps = _shard_inputs(inputs)
    res = run_bass_kernel_spmd(nc, in_maps, core_ids=list(range(N_CORES)), **spmd_kwargs)
    out = np.empty((B, T, C), np.float32)
    for c in range(N_CORES):
        b, half = c // 2, c % 2
        out[b, half * TQ:(half + 1) * TQ] = \
            np.asarray(res.results[c]["out"]).astype(np.float32)
    return out, res


def kernel(**inputs) -> np.ndarray:
    out, _ = _run(inputs)
    return out

